# revision 19
# baseline (speedup 1.0000x reference)
"""Trainium2 Bass kernel: 2-layer BiLSTM classifier (B=32, I=128, T=512, H=512, O=10).

Sharding: data-parallel over batch across 8 NeuronCores (b=4 per core); both
directions and both layers run locally per core; host splits/concats.

Per layer per direction:
  xp = Wih' @ input + b'                          (bulk projection -> DRAM)
  per step: g = xp_t + Whh' @ H_{t-1}             (xp DMA'd into PSUM, PE accumulates)
  all-gate tanh trick (host pre-scales i,f,o rows by 0.5):
      t = tanh(g);  for i/f/o blocks t = 2*sigmoid(a)-1; for g block t = tanh(a)
      C_t = 0.5*(t_f+1)*C_{t-1} + (t_i+1)*t_g     [C = 2c]
      H_t = (t_o+1)*tanh(0.5*C_t)                 [H = 2h]
  The H=2h factor is absorbed into Whh/Wih1/Wlin columns (x0.5 host side).
Gate-block order is (i, g, f, o); block = PSUM bank.

PSUM: two parity tiles (gA/gB) of 4 banks each; fwd occupies partitions 0:4,
bwd partitions 4:8 of the same banks.  xp rows for step u+1 are DMA'd into
parity (u+1)%2 while step u computes, and all Whh matmuls run start=False,
accumulating onto the DMA'd xp.  Hidden states stay in SBUF (bf16) and feed
layer-1's projection directly; matmul operands are bf16 (psum accum f32).
"""

import numpy as np

B_FULL, I_IN, T, H, O = 32, 128, 512, 512, 10
NCORES = 8
B = B_FULL // NCORES      # 4
G4 = 4 * H                # 2048
NB = 4                    # gate banks
TBLK = 32                 # projection t-block
NTBLK = T // TBLK         # 16
PRO = 16                  # python-unrolled prologue steps
UNROLL = 16               # steps per For_i iteration
XPAD = UNROLL * B         # xp scratch row padding (prefetch slack both ends)

_CACHE = {}


def _build_nc():
    import concourse.bass as bass
    import concourse.mybir as mybir
    import concourse.tile as tile
    from concourse import bacc
    from concourse.bass import ds

    F32 = mybir.dt.float32
    F32R = mybir.dt.float32r
    BF16 = mybir.dt.bfloat16
    AF = mybir.ActivationFunctionType
    OP = mybir.AluOpType

    nc = bacc.Bacc("TRN2", target_bir_lowering=False, debug=False, num_devices=NCORES)

    # ---------------- I/O ----------------
    xT_d = nc.dram_tensor("xT", [I_IN, T * B], BF16, kind="ExternalInput")
    wih0_d = {d: nc.dram_tensor(f"wih0{d}", [I_IN, G4], BF16, kind="ExternalInput") for d in "fr"}
    wih1_d = {d: nc.dram_tensor(f"wih1{d}", [128, 8 * G4], BF16, kind="ExternalInput") for d in "fr"}
    whh_d = {(l, d): nc.dram_tensor(f"whh{l}{d}", [128, 4 * G4], BF16, kind="ExternalInput")
             for l in range(2) for d in "fr"}
    b_d = {(l, d): nc.dram_tensor(f"b{l}{d}", [1, G4], F32R, kind="ExternalInput")
           for l in range(2) for d in "fr"}
    wlin_d = nc.dram_tensor("wlin", [128, 8 * O], F32R, kind="ExternalInput")
    blin_d = nc.dram_tensor("blin", [1, O], F32R, kind="ExternalInput")
    ones_d = nc.dram_tensor("ones", [1, 128], F32R, kind="ExternalInput")
    i4_d = nc.dram_tensor("i4", [B, B], F32R, kind="ExternalInput")
    i8_d = nc.dram_tensor("i8", [8, 36], F32R, kind="ExternalInput")
    out_d = nc.dram_tensor("out", [O, B], F32, kind="ExternalOutput")

    # DRAM scratch (rows padded by XPAD at both ends for prefetch slack)
    xp_dram = {(l, d): nc.dram_tensor(f"xp{l}{d}", [(T + 2 * UNROLL) * B, G4], F32R)
               for l in range(2) for d in "fr"}

    with tile.TileContext(nc) as tc:
        import contextlib

        ctx = contextlib.ExitStack()
        sbuf = ctx.enter_context(tc.tile_pool(name="sbuf", bufs=1))
        psum = ctx.enter_context(tc.tile_pool(name="psum", bufs=1, space="PSUM"))
        xpp = ctx.enter_context(tc.tile_pool(name="xpp", bufs=2))   # brow slots
        tsp = ctx.enter_context(tc.tile_pool(name="tsp", bufs=2))   # tsb slots
        evp = ctx.enter_context(tc.tile_pool(name="evp", bufs=3))   # 2KB slots
        smal = ctx.enter_context(tc.tile_pool(name="smal", bufs=1))
        wpp = ctx.enter_context(tc.tile_pool(name="wpp", bufs=2))   # 8KB slots
        xtp = ctx.enter_context(tc.tile_pool(name="xtp", bufs=2))   # xt slots

        with ctx:
            # ---------- static tiles ----------
            ones_t = sbuf.tile([1, 128], F32R)
            nc.sync.dma_start(out=ones_t, in_=ones_d.ap())
            i4_t = sbuf.tile([B, B], F32R)
            nc.sync.dma_start(out=i4_t, in_=i4_d.ap())
            i8_t = sbuf.tile([8, 36], F32R)
            nc.sync.dma_start(out=i8_t, in_=i8_d.ap())
            i4w_t = sbuf.tile([36, B], F32R, name="i4w")
            nc.sync.dma_start(out=i4w_t[0:B, :], in_=i4_d.ap())
            nc.sync.dma_start(out=i4w_t[32:36, :], in_=i4_d.ap())
            blin_t = sbuf.tile([1, O], F32R)
            nc.sync.dma_start(out=blin_t, in_=blin_d.ap())
            wlin_t = sbuf.tile([128, 8 * O], F32R)
            nc.sync.dma_start(out=wlin_t, in_=wlin_d.ap())

            # hring: [128, k(4), slot(8), b(4)] bf16 per dir
            hring = {d: sbuf.tile([128, 4, UNROLL, B], BF16, name=f"hring_{d}") for d in "fr"}
            whh_t = {d: sbuf.tile([128, 4 * G4], BF16, name=f"whh_{d}") for d in "fr"}
            # hidden states of layer 0, SBUF-resident: [128, k(4), T*B] bf16
            hs_sb = {d: sbuf.tile([128, 4, T * B], BF16, name=f"hs_{d}") for d in "fr"}
            c_t = {d: sbuf.tile([36, H], F32, name=f"c_{d}") for d in "fr"}
            pooled = {d: sbuf.tile([128, 4 * B], F32, name=f"pooled_{d}") for d in "fr"}

            ROW = {"f": 0, "r": 32}     # partition offset per direction
            TRC = {"f": 0, "r": 16}    # trv column offset per direction

            # ================= projection =================
            def projection(layer):
                brow = {}
                for d in "fr":
                    brow[d] = xpp.tile([1, G4], F32R, tag=f"xpc{d}", name=f"brow{d}", bufs=1)
                    nc.sync.dma_start(out=brow[d], in_=b_d[(layer, d)].ap())
                if layer == 0:
                    wih0_t = wpp.tile([I_IN, 2 * G4], BF16, tag="wp0", bufs=1)
                    for di, d in enumerate("fr"):
                        nc.sync.dma_start(
                            out=wih0_t[:, G4 * di : G4 * (di + 1)], in_=wih0_d[d].ap())
                for di, d in enumerate("fr"):
                    for blk in range(NB):
                        if layer == 1:
                            wt = wpp.tile([128, 8 * 512], BF16, tag="wp")
                            for k in range(8):
                                nc.sync.dma_start(
                                    out=wt[:, 512 * k : 512 * (k + 1)],
                                    in_=wih1_d[d].ap()[:, G4 * k + 512 * blk : G4 * k + 512 * blk + 512],
                                )
                        for t0 in range(NTBLK):
                            pp = psum.tile([128, 512], F32, tag=("gA" if t0 % 2 == 0 else "gB"), name="pp")
                            nc.tensor.matmul(
                                pp, lhsT=ones_t,
                                rhs=brow[d][:, 512 * blk : 512 * (blk + 1)],
                                start=True, stop=False)
                            if layer == 0:
                                xt = xtp.tile([I_IN, TBLK * B], BF16, tag="xt")
                                nc.sync.dma_start(
                                    out=xt,
                                    in_=xT_d.ap()[:, TBLK * B * t0 : TBLK * B * (t0 + 1)])
                                nc.tensor.matmul(
                                    pp, lhsT=xt,
                                    rhs=wih0_t[:, G4 * di + 512 * blk : G4 * di + 512 * blk + 512],
                                    start=False, stop=True)
                            else:
                                for k in range(8):
                                    dd = "f" if k < 4 else "r"
                                    nc.tensor.matmul(
                                        pp,
                                        lhsT=hs_sb[dd][:, k % 4, TBLK * B * t0 : TBLK * B * t0 + TBLK * B],
                                        rhs=wt[:, 512 * k : 512 * (k + 1)],
                                        start=False, stop=(k == 7))
                            ev = evp.tile([128, 512], F32R, tag="ev")
                            nc.scalar.activation(ev, pp, AF.Identity)
                            nc.sync.dma_start(
                                out=xp_dram[(layer, d)].ap()[
                                    XPAD + TBLK * B * t0 : XPAD + TBLK * B * (t0 + 1),
                                    512 * blk : 512 * (blk + 1)],
                                in_=ev)

            # ================= recurrence =================
            # Two-direction software pipeline: per step the PE FIFO sees
            # [fwd MM-group][bwd tail(t-1) transposes][bwd MM-group][fwd tail]
            # so PE never stalls on a gate chain.
            tsb_cur = {}
            gps = {}

            def toff_of(d, i, u):
                if i is None:
                    t = u
                    tt = t if d == "f" else T - 1 - t
                    return B * tt
                if d == "f":
                    return i * (UNROLL * B) + u * B
                return i * (-UNROLL * B) + (T - 1 - u) * B

            def emit_xp_inject(layer, i, u, is_t0=False):
                # Stage xp rows for step (i, u) into SBUF (both dirs stacked),
                # then one PE inject matmul per bank covers fwd+bwd at once.
                buf = gps[u % 2]
                xpc = xpp.tile([8, G4], F32R, tag="xpc", name="xpc", bufs=2)
                for d in "fr":
                    toff = toff_of(d, i, u)
                    if isinstance(toff, int):
                        src = xp_dram[(layer, d)].ap()[XPAD + toff : XPAD + toff + B, :]
                    else:
                        src = xp_dram[(layer, d)].ap()[ds(XPAD + toff, B), :]
                    r0x = 0 if d == "f" else B
                    nc.sync.dma_start(out=xpc[r0x : r0x + B, :], in_=src)
                for blk in range(NB):
                    nc.tensor.matmul(
                        buf[0:36, blk, :],
                        lhsT=i8_t,
                        rhs=xpc[:, 512 * blk : 512 * (blk + 1)],
                        start=True, stop=is_t0, skip_group_check=True)

            def emit_mms(layer, d, is_t0, u, pslot):
                buf = gps[u % 2]
                r0 = ROW[d]
                tsb = tsp.tile([36, G4], F32, tag=f"tsb{d}", name=f"tsb{d}")
                tsb_cur[d] = tsb
                if not is_t0:
                    for blk in range(NB):
                        for k in range(4):
                            nc.tensor.matmul(
                                buf[r0 : r0 + B, blk, :],
                                lhsT=hring[d][:, k, pslot, :],
                                rhs=whh_t[d][:, G4 * k + 512 * blk : G4 * k + 512 * blk + 512],
                                start=False, stop=(k == 3), skip_group_check=True)
                nc.scalar.activation(
                    tsb[r0 : r0 + B, 0:1024].rearrange("b (n g) -> b n g", n=2),
                    buf[r0 : r0 + B, 0:2, :], AF.Tanh)
                nc.scalar.activation(
                    tsb[r0 : r0 + B, 1024:2048].rearrange("b (n g) -> b n g", n=2),
                    buf[r0 : r0 + B, 2:4, :], AF.Tanh)

            h_cur = {}

            def emit_tail_math(layer, d, u):
                r0 = ROW[d]
                tsb = tsb_cur[d]
                rr = slice(r0, r0 + B)
                a_t = smal.tile([36, H], F32, tag="a", name="a_t")
                nc.vector.scalar_tensor_tensor(
                    out=a_t[rr], in0=tsb[rr, 0:512], scalar=1.0,
                    in1=tsb[rr, 512:1024], op0=OP.add, op1=OP.mult)
                bb_t = smal.tile([36, H], F32, tag="bb", name="bb_t")
                nc.vector.scalar_tensor_tensor(
                    out=bb_t[rr], in0=tsb[rr, 1024:1536], scalar=1.0,
                    in1=c_t[d][rr], op0=OP.add, op1=OP.mult)
                tch = smal.tile([36, H], F32, tag="tc", name="tch")
                h_t = smal.tile([36, H], F32R, tag="h", name="h_t")
                h_cur[d] = h_t
                for hh in range(2):
                    sl = slice(256 * hh, 256 * hh + 256)
                    so = slice(1536 + 256 * hh, 1536 + 256 * hh + 256)
                    nc.vector.scalar_tensor_tensor(
                        out=c_t[d][rr, sl], in0=bb_t[rr, sl], scalar=0.5,
                        in1=a_t[rr, sl], op0=OP.mult, op1=OP.add)
                    nc.scalar.activation(tch[rr, sl], c_t[d][rr, sl], AF.Tanh, scale=0.5)
                    nc.vector.scalar_tensor_tensor(
                        out=h_t[rr, sl], in0=tsb[rr, so], scalar=1.0,
                        in1=tch[rr, sl], op0=OP.add, op1=OP.mult)

            def emit_tail_tr(layer, d, u, slot):
                buf = gps[u % 2]
                r0 = ROW[d]
                c0 = TRC[d]
                rr = slice(r0, r0 + B)
                h_t = h_cur[d]
                trv = buf.bitcast(F32R)
                for k in range(4):
                    nc.tensor.transpose(
                        trv[:, 0, c0 + B * k : c0 + B * (k + 1)],
                        h_t[rr, 128 * k : 128 * (k + 1)],
                        i4w_t[rr])
                # next-step stationary (bf16 cast) + layer-0 hs archive ring
                nc.vector.tensor_copy(
                    hring[d][:, :, slot, :],
                    buf.bitcast(F32)[:, 0, c0 : c0 + 16].rearrange("p (k b) -> p k b", k=4))
                if layer == 1:
                    nc.vector.tensor_tensor(
                        out=pooled[d], in0=pooled[d],
                        in1=buf.bitcast(F32)[:, 0, c0 : c0 + 16],
                        op=OP.add)

            def flush_half(t_first, half):
                # archive 8 completed h-slots [half*8 : half*8+8] into hs_sb.
                # t_first = global step index of slot half*8 (int or reg expr).
                HB = 8 * B  # 32 cols per half
                s0 = half * 8
                d = "f"
                base = B * t_first if not isinstance(t_first, int) else B * t_first
                srcap = bass.AP(
                    tensor=hring[d].tensor,
                    offset=hring[d][:, 0, s0, 0:B].offset,
                    ap=[list(hring[d].ap[0]), [UNROLL * B, 4], [1, HB]],
                )
                if isinstance(t_first, int):
                    dst = hs_sb[d][:, :, B * t_first : B * t_first + HB]
                else:
                    dst = hs_sb[d][:, :, ds(B * t_first, HB)]
                nc.sync.dma_start(out=dst, in_=srcap)
                d = "r"
                rbase = B * (T - 8 - t_first)
                for k in range(4):
                    srcap = bass.AP(
                        tensor=hring[d].tensor,
                        offset=hring[d][:, k, s0 + 7, 0:B].offset,
                        ap=[list(hring[d].ap[0]), [-B, 8], [1, B]],
                    )
                    if isinstance(t_first, int):
                        dst = hs_sb[d][:, k, rbase : rbase + HB]
                    else:
                        dst = hs_sb[d][:, k, ds(rbase, HB)]
                    nc.sync.dma_start(out=dst, in_=srcap)

            def recurrence(layer):
                gps[0] = psum.tile([128, NB, 512], F32, tag="gA", name="gA")
                gps[1] = psum.tile([128, NB, 512], F32, tag="gB", name="gB")
                for d in "fr":
                    nc.vector.memset(c_t[d], 0.0)
                    if layer == 1:
                        nc.vector.memset(pooled[d], 0.0)

                def steps(i, urange):
                    prev_u = None
                    for u in urange:
                        is0 = (i is None and u == 0)
                        if is0:
                            emit_xp_inject(layer, i, u, is_t0=True)
                        if prev_u is not None:
                            emit_tail_math(layer, "r", prev_u)
                        emit_mms(layer, "f", is0, u, (u - 1) % UNROLL)
                        if prev_u is not None:
                            emit_tail_tr(layer, "r", prev_u, prev_u % UNROLL)
                        emit_tail_math(layer, "f", u)
                        emit_mms(layer, "r", is0, u, (u - 1) % UNROLL)
                        emit_xp_inject(layer, i, u + 1)
                        emit_tail_tr(layer, "f", u, u % UNROLL)
                        if layer == 0 and u == 4 and i is not None:
                            flush_half(i * UNROLL - 8, 1)
                        if layer == 0 and u == 12:
                            flush_half(0 if i is None else i * UNROLL, 0)
                        prev_u = u
                    emit_tail_math(layer, "r", prev_u)
                    emit_tail_tr(layer, "r", prev_u, prev_u % UNROLL)

                steps(None, range(PRO))
                with tc.For_i(1, T // UNROLL) as i:
                    steps(i, range(UNROLL))
                if layer == 0:
                    flush_half(T - 8, 1)

            # ================= run =================
            for d in "fr":
                nc.sync.dma_start(out=whh_t[d], in_=whh_d[(0, d)].ap())
            projection(0)
            recurrence(0)
            projection(1)
            for d in "fr":
                nc.sync.dma_start(out=whh_t[d], in_=whh_d[(1, d)].ap())
            recurrence(1)

            # ---------- final linear ----------
            plr = {d: sbuf.tile([128, 4 * B], F32R, name=f"plr_{d}") for d in "fr"}
            for d in "fr":
                nc.vector.tensor_copy(plr[d], pooled[d])
            fin_ps = psum.tile([O, B], F32, tag="gA", name="fin_ps")
            nc.tensor.matmul(fin_ps, lhsT=blin_t, rhs=ones_t[:, 0:B],
                             start=True, stop=False)
            for k in range(8):
                dd = "f" if k < 4 else "r"
                nc.tensor.matmul(
                    fin_ps,
                    lhsT=wlin_t[:, O * k : O * (k + 1)],
                    rhs=plr[dd][:, B * (k % 4) : B * (k % 4 + 1)],
                    start=False, stop=(k == 7))
            fin_sb = sbuf.tile([O, B], F32)
            nc.scalar.copy(fin_sb, fin_ps)
            nc.sync.dma_start(out=out_d.ap(), in_=fin_sb)

    nc.compile()
    return nc


# ======================= host side =======================

def _prep_weights(inputs):
    import ml_dtypes
    f32 = np.float32
    bf16 = ml_dtypes.bfloat16
    perm = np.concatenate([np.arange(0, 512), np.arange(1024, 1536),
                           np.arange(512, 1024), np.arange(1536, 2048)])
    rs = np.ones(G4, f32) * 0.5
    rs[512:1024] = 1.0

    def whh_dev(W):
        Wp = (W[perm] * rs[:, None] * 0.5).astype(f32)     # [2048, 512]
        return np.ascontiguousarray(
            Wp.T.reshape(4, 128, G4).transpose(1, 0, 2).reshape(128, 4 * G4)).astype(bf16)

    def wih1_dev(W):
        Wp = (W[perm] * rs[:, None] * 0.5).astype(f32)     # [2048, 1024]
        return np.ascontiguousarray(
            Wp.T.reshape(8, 128, G4).transpose(1, 0, 2).reshape(128, 8 * G4)).astype(bf16)

    out = {}
    for d in "fr":
        out[f"wih0{d}"] = np.ascontiguousarray(
            (inputs[f"Wih0{d}"][perm] * rs[:, None]).astype(f32).T).astype(bf16)
        out[f"whh0{d}"] = whh_dev(inputs[f"Whh0{d}"])
        out[f"b0{d}"] = (inputs[f"b0{d}"][perm] * rs).astype(f32)[None, :]
        out[f"wih1{d}"] = wih1_dev(inputs[f"Wih1{d}"])
        out[f"whh1{d}"] = whh_dev(inputs[f"Whh1{d}"])
        out[f"b1{d}"] = (inputs[f"b1{d}"][perm] * rs).astype(f32)[None, :]
    wl = (inputs["Wlin"] * (0.5 / T)).astype(f32)           # [10, 1024]
    out["wlin"] = np.ascontiguousarray(
        wl.T.reshape(8, 128, O).transpose(1, 0, 2).reshape(128, 8 * O))
    out["blin"] = inputs["blin"].astype(f32)[None, :]
    out["ones"] = np.ones((1, 128), f32)
    out["i4"] = np.eye(B, dtype=f32)
    p836 = np.zeros((8, 36), f32)
    p836[:4, :4] = np.eye(4)
    p836[4:, 32:] = np.eye(4)
    out["i8"] = p836
    return out


def make_in_maps(inputs):
    import ml_dtypes
    shared = _prep_weights(inputs)
    x = np.asarray(inputs["x"], dtype=np.float32)           # [32, 128, 512]
    in_maps = []
    for c in range(NCORES):
        xs = x[B * c : B * (c + 1)]                         # [4, 128, 512]
        m = dict(shared)
        m["xT"] = np.ascontiguousarray(
            xs.transpose(1, 2, 0).reshape(I_IN, T * B)).astype(ml_dtypes.bfloat16)
        in_maps.append(m)
    return in_maps


def kernel(**inputs):
    from concourse.bass_utils import run_bass_kernel_spmd

    if "nc" not in _CACHE:
        _CACHE["nc"] = _build_nc()
    nc = _CACHE["nc"]

    in_maps = make_in_maps(inputs)
    res = run_bass_kernel_spmd(nc, in_maps, core_ids=list(range(NCORES)))
    out = np.zeros((B_FULL, O), np.float32)
    for c in range(NCORES):
        out[B * c : B * (c + 1)] = res.results[c]["out"].T
    return out


# revision 20
# speedup vs baseline: 1.0069x; 1.0069x over previous
"""Trainium2 Bass kernel: 2-layer BiLSTM classifier (B=32, I=128, T=512, H=512, O=10).

Sharding: data-parallel over batch across 8 NeuronCores (b=4 per core); both
directions and both layers run locally per core; host splits/concats.

Per layer per direction:
  xp = Wih' @ input + b'                          (bulk projection -> DRAM)
  per step: g = xp_t + Whh' @ H_{t-1}             (xp DMA'd into PSUM, PE accumulates)
  all-gate tanh trick (host pre-scales i,f,o rows by 0.5):
      t = tanh(g);  for i/f/o blocks t = 2*sigmoid(a)-1; for g block t = tanh(a)
      C_t = 0.5*(t_f+1)*C_{t-1} + (t_i+1)*t_g     [C = 2c]
      H_t = (t_o+1)*tanh(0.5*C_t)                 [H = 2h]
  The H=2h factor is absorbed into Whh/Wih1/Wlin columns (x0.5 host side).
Gate-block order is (f, o, i, g); block = PSUM bank.

PSUM: two parity tiles (gA/gB) of 4 banks each; fwd occupies partitions 0:4,
bwd partitions 4:8 of the same banks.  xp rows for step u+1 are DMA'd into
parity (u+1)%2 while step u computes, and all Whh matmuls run start=False,
accumulating onto the DMA'd xp.  Hidden states stay in SBUF (bf16) and feed
layer-1's projection directly; matmul operands are bf16 (psum accum f32).
"""

import numpy as np

B_FULL, I_IN, T, H, O = 32, 128, 512, 512, 10
NCORES = 8
B = B_FULL // NCORES      # 4
G4 = 4 * H                # 2048
NB = 4                    # gate banks
TBLK = 32                 # projection t-block
NTBLK = T // TBLK         # 16
PRO = 16                  # python-unrolled prologue steps
UNROLL = 16               # steps per For_i iteration
XPAD = UNROLL * B         # xp scratch row padding (prefetch slack both ends)

_CACHE = {}


def _build_nc():
    import concourse.bass as bass
    import concourse.mybir as mybir
    import concourse.tile as tile
    from concourse import bacc
    from concourse.bass import ds

    F32 = mybir.dt.float32
    F32R = mybir.dt.float32r
    BF16 = mybir.dt.bfloat16
    AF = mybir.ActivationFunctionType
    OP = mybir.AluOpType

    nc = bacc.Bacc("TRN2", target_bir_lowering=False, debug=False, num_devices=NCORES)

    # ---------------- I/O ----------------
    xT_d = nc.dram_tensor("xT", [I_IN, T * B], BF16, kind="ExternalInput")
    wih0_d = {d: nc.dram_tensor(f"wih0{d}", [I_IN, G4], BF16, kind="ExternalInput") for d in "fr"}
    wih1_d = {d: nc.dram_tensor(f"wih1{d}", [128, 8 * G4], BF16, kind="ExternalInput") for d in "fr"}
    whh_d = {(l, d): nc.dram_tensor(f"whh{l}{d}", [128, 4 * G4], BF16, kind="ExternalInput")
             for l in range(2) for d in "fr"}
    b_d = {(l, d): nc.dram_tensor(f"b{l}{d}", [1, G4], F32R, kind="ExternalInput")
           for l in range(2) for d in "fr"}
    wlin_d = nc.dram_tensor("wlin", [128, 8 * O], F32R, kind="ExternalInput")
    blin_d = nc.dram_tensor("blin", [1, O], F32R, kind="ExternalInput")
    ones_d = nc.dram_tensor("ones", [1, 128], F32R, kind="ExternalInput")
    i4_d = nc.dram_tensor("i4", [B, B], F32R, kind="ExternalInput")
    i8_d = nc.dram_tensor("i8", [8, 36], F32R, kind="ExternalInput")
    i4b_d = nc.dram_tensor("i4b", [B, B], BF16, kind="ExternalInput")
    out_d = nc.dram_tensor("out", [O, B], F32, kind="ExternalOutput")

    # DRAM scratch (rows padded by XPAD at both ends for prefetch slack)
    xp_dram = {(l, d): nc.dram_tensor(f"xp{l}{d}", [(T + 2 * UNROLL) * B, G4], F32R)
               for l in range(2) for d in "fr"}

    with tile.TileContext(nc) as tc:
        import contextlib

        ctx = contextlib.ExitStack()
        sbuf = ctx.enter_context(tc.tile_pool(name="sbuf", bufs=1))
        psum = ctx.enter_context(tc.tile_pool(name="psum", bufs=1, space="PSUM"))
        xpp = ctx.enter_context(tc.tile_pool(name="xpp", bufs=2))   # brow slots
        tsp = ctx.enter_context(tc.tile_pool(name="tsp", bufs=2))   # tsb slots
        evp = ctx.enter_context(tc.tile_pool(name="evp", bufs=3))   # 2KB slots
        smal = ctx.enter_context(tc.tile_pool(name="smal", bufs=1))
        wpp = ctx.enter_context(tc.tile_pool(name="wpp", bufs=2))   # 8KB slots
        xtp = ctx.enter_context(tc.tile_pool(name="xtp", bufs=2))   # xt slots

        with ctx:
            # ---------- static tiles ----------
            ones_t = sbuf.tile([1, 128], F32R)
            nc.sync.dma_start(out=ones_t, in_=ones_d.ap())
            i4_t = sbuf.tile([B, B], F32R)
            nc.sync.dma_start(out=i4_t, in_=i4_d.ap())
            i8_t = sbuf.tile([8, 36], F32R)
            nc.sync.dma_start(out=i8_t, in_=i8_d.ap())
            i4w_t = sbuf.tile([36, B], BF16, name="i4w")
            nc.sync.dma_start(out=i4w_t[0:B, :], in_=i4b_d.ap())
            nc.sync.dma_start(out=i4w_t[32:36, :], in_=i4b_d.ap())
            blin_t = sbuf.tile([1, O], F32R)
            nc.sync.dma_start(out=blin_t, in_=blin_d.ap())
            wlin_t = sbuf.tile([128, 8 * O], F32R)
            nc.sync.dma_start(out=wlin_t, in_=wlin_d.ap())

            # hring: [128, k(4), slot(8), b(4)] bf16 per dir
            hring = {d: sbuf.tile([128, 4, UNROLL, B], BF16, name=f"hring_{d}") for d in "fr"}
            whh_t = {d: sbuf.tile([128, 4 * G4], BF16, name=f"whh_{d}") for d in "fr"}
            # hidden states of layer 0, SBUF-resident: [128, k(4), T*B] bf16
            hs_sb = {d: sbuf.tile([128, 4, T * B], BF16, name=f"hs_{d}") for d in "fr"}
            c_t = {d: sbuf.tile([36, H], F32, name=f"c_{d}") for d in "fr"}
            pooled = {d: sbuf.tile([128, 4 * B], F32, name=f"pooled_{d}") for d in "fr"}

            ROW = {"f": 0, "r": 32}     # partition offset per direction
            TRC = {"f": 0, "r": 16}    # trv column offset per direction

            # ================= projection =================
            def projection(layer):
                brow = {}
                for d in "fr":
                    brow[d] = xpp.tile([1, G4], F32R, tag=f"xpc{d}", name=f"brow{d}", bufs=1)
                    nc.sync.dma_start(out=brow[d], in_=b_d[(layer, d)].ap())
                if layer == 0:
                    wih0_t = wpp.tile([I_IN, 2 * G4], BF16, tag="wp0", bufs=1)
                    for di, d in enumerate("fr"):
                        nc.sync.dma_start(
                            out=wih0_t[:, G4 * di : G4 * (di + 1)], in_=wih0_d[d].ap())
                for di, d in enumerate("fr"):
                    for blk in range(NB):
                        if layer == 1:
                            wt = wpp.tile([128, 8 * 512], BF16, tag="wp")
                            for k in range(8):
                                nc.sync.dma_start(
                                    out=wt[:, 512 * k : 512 * (k + 1)],
                                    in_=wih1_d[d].ap()[:, G4 * k + 512 * blk : G4 * k + 512 * blk + 512],
                                )
                        for t0 in range(NTBLK):
                            pp = psum.tile([128, 512], F32, tag=("gA" if t0 % 2 == 0 else "gB"), name="pp")
                            nc.tensor.matmul(
                                pp, lhsT=ones_t,
                                rhs=brow[d][:, 512 * blk : 512 * (blk + 1)],
                                start=True, stop=False)
                            if layer == 0:
                                xt = xtp.tile([I_IN, TBLK * B], BF16, tag="xt")
                                nc.sync.dma_start(
                                    out=xt,
                                    in_=xT_d.ap()[:, TBLK * B * t0 : TBLK * B * (t0 + 1)])
                                nc.tensor.matmul(
                                    pp, lhsT=xt,
                                    rhs=wih0_t[:, G4 * di + 512 * blk : G4 * di + 512 * blk + 512],
                                    start=False, stop=True)
                            else:
                                for k in range(8):
                                    dd = "f" if k < 4 else "r"
                                    nc.tensor.matmul(
                                        pp,
                                        lhsT=hs_sb[dd][:, k % 4, TBLK * B * t0 : TBLK * B * t0 + TBLK * B],
                                        rhs=wt[:, 512 * k : 512 * (k + 1)],
                                        start=False, stop=(k == 7))
                            ev = evp.tile([128, 512], F32R, tag="ev")
                            nc.scalar.activation(ev, pp, AF.Identity)
                            nc.sync.dma_start(
                                out=xp_dram[(layer, d)].ap()[
                                    XPAD + TBLK * B * t0 : XPAD + TBLK * B * (t0 + 1),
                                    512 * blk : 512 * (blk + 1)],
                                in_=ev)

            # ================= recurrence =================
            # Two-direction software pipeline: per step the PE FIFO sees
            # [fwd MM-group][bwd tail(t-1) transposes][bwd MM-group][fwd tail]
            # so PE never stalls on a gate chain.
            tsb_cur = {}
            gps = {}

            def toff_of(d, i, u):
                if i is None:
                    t = u
                    tt = t if d == "f" else T - 1 - t
                    return B * tt
                if d == "f":
                    return i * (UNROLL * B) + u * B
                return i * (-UNROLL * B) + (T - 1 - u) * B

            def emit_xp_inject(layer, i, u, is_t0=False):
                # Stage xp rows for step (i, u) into SBUF (both dirs stacked),
                # then one PE inject matmul per bank covers fwd+bwd at once.
                buf = gps[u % 2]
                xpc = xpp.tile([8, G4], F32R, tag="xpc", name="xpc", bufs=2)
                for d in "fr":
                    toff = toff_of(d, i, u)
                    if isinstance(toff, int):
                        src = xp_dram[(layer, d)].ap()[XPAD + toff : XPAD + toff + B, :]
                    else:
                        src = xp_dram[(layer, d)].ap()[ds(XPAD + toff, B), :]
                    r0x = 0 if d == "f" else B
                    nc.sync.dma_start(out=xpc[r0x : r0x + B, :], in_=src)
                for blk in range(NB):
                    nc.tensor.matmul(
                        buf[0:36, blk, :],
                        lhsT=i8_t,
                        rhs=xpc[:, 512 * blk : 512 * (blk + 1)],
                        start=True, stop=is_t0, skip_group_check=True)

            def emit_mms(layer, d, is_t0, u, pslot):
                buf = gps[u % 2]
                r0 = ROW[d]
                tsb = tsp.tile([36, G4], BF16, tag=f"tsb{d}", name=f"tsb{d}")
                tsb_cur[d] = tsb
                if not is_t0:
                    for blk in range(NB):
                        for k in range(4):
                            nc.tensor.matmul(
                                buf[r0 : r0 + B, blk, :],
                                lhsT=hring[d][:, k, pslot, :],
                                rhs=whh_t[d][:, G4 * k + 512 * blk : G4 * k + 512 * blk + 512],
                                start=False, stop=(k == 3), skip_group_check=True)
                nc.scalar.activation(
                    tsb[r0 : r0 + B, 0:1024].rearrange("b (n g) -> b n g", n=2),
                    buf[r0 : r0 + B, 0:2, :], AF.Tanh)
                nc.scalar.activation(
                    tsb[r0 : r0 + B, 1024:2048].rearrange("b (n g) -> b n g", n=2),
                    buf[r0 : r0 + B, 2:4, :], AF.Tanh)

            h_cur = {}

            def emit_tail_math(layer, d, u):
                r0 = ROW[d]
                tsb = tsb_cur[d]
                rr = slice(r0, r0 + B)
                bb_t = smal.tile([36, H], F32, tag="bb", name="bb_t")
                nc.vector.scalar_tensor_tensor(
                    out=bb_t[rr], in0=tsb[rr, 0:512], scalar=1.0,
                    in1=c_t[d][rr], op0=OP.add, op1=OP.mult)
                a_t = smal.tile([36, H], BF16, tag="a", name="a_t")
                nc.vector.scalar_tensor_tensor(
                    out=a_t[rr], in0=tsb[rr, 1024:1536], scalar=1.0,
                    in1=tsb[rr, 1536:2048], op0=OP.add, op1=OP.mult)
                nc.vector.scalar_tensor_tensor(
                    out=c_t[d][rr], in0=bb_t[rr], scalar=0.5,
                    in1=a_t[rr], op0=OP.mult, op1=OP.add)
                tch = smal.tile([36, H], BF16, tag="tc", name="tch")
                nc.scalar.activation(tch[rr], c_t[d][rr], AF.Tanh, scale=0.5)
                h_t = smal.tile([36, H], BF16, tag="h", name="h_t")
                h_cur[d] = h_t
                nc.vector.scalar_tensor_tensor(
                    out=h_t[rr], in0=tsb[rr, 512:1024], scalar=1.0,
                    in1=tch[rr], op0=OP.add, op1=OP.mult)

            def emit_tail_tr(layer, d, u, slot):
                buf = gps[u % 2]
                r0 = ROW[d]
                c0 = TRC[d]
                rr = slice(r0, r0 + B)
                h_t = h_cur[d]
                trv = buf.bitcast(BF16)
                for k in range(4):
                    nc.tensor.transpose(
                        trv[:, 0, c0 + B * k : c0 + B * (k + 1)],
                        h_t[rr, 128 * k : 128 * (k + 1)],
                        i4w_t[rr])
                # next-step stationary + layer-0 hs archive ring
                nc.vector.tensor_copy(
                    hring[d][:, :, slot, :],
                    trv[:, 0, c0 : c0 + 16].rearrange("p (k b) -> p k b", k=4))
                if layer == 1:
                    nc.vector.tensor_tensor(
                        out=pooled[d], in0=pooled[d],
                        in1=trv[:, 0, c0 : c0 + 16],
                        op=OP.add)

            def flush_half(t_first, half):
                # archive 8 completed h-slots [half*8 : half*8+8] into hs_sb.
                # t_first = global step index of slot half*8 (int or reg expr).
                HB = 8 * B  # 32 cols per half
                s0 = half * 8
                d = "f"
                base = B * t_first if not isinstance(t_first, int) else B * t_first
                srcap = bass.AP(
                    tensor=hring[d].tensor,
                    offset=hring[d][:, 0, s0, 0:B].offset,
                    ap=[list(hring[d].ap[0]), [UNROLL * B, 4], [1, HB]],
                )
                if isinstance(t_first, int):
                    dst = hs_sb[d][:, :, B * t_first : B * t_first + HB]
                else:
                    dst = hs_sb[d][:, :, ds(B * t_first, HB)]
                nc.sync.dma_start(out=dst, in_=srcap)
                d = "r"
                rbase = B * (T - 8 - t_first)
                for k in range(4):
                    srcap = bass.AP(
                        tensor=hring[d].tensor,
                        offset=hring[d][:, k, s0 + 7, 0:B].offset,
                        ap=[list(hring[d].ap[0]), [-B, 8], [1, B]],
                    )
                    if isinstance(t_first, int):
                        dst = hs_sb[d][:, k, rbase : rbase + HB]
                    else:
                        dst = hs_sb[d][:, k, ds(rbase, HB)]
                    nc.sync.dma_start(out=dst, in_=srcap)

            def recurrence(layer):
                gps[0] = psum.tile([128, NB, 512], F32, tag="gA", name="gA")
                gps[1] = psum.tile([128, NB, 512], F32, tag="gB", name="gB")
                for d in "fr":
                    nc.vector.memset(c_t[d], 0.0)
                    if layer == 1:
                        nc.vector.memset(pooled[d], 0.0)

                def steps(i, urange):
                    prev_u = None
                    for u in urange:
                        is0 = (i is None and u == 0)
                        if is0:
                            emit_xp_inject(layer, i, u, is_t0=True)
                        if prev_u is not None:
                            emit_tail_math(layer, "r", prev_u)
                        emit_mms(layer, "f", is0, u, (u - 1) % UNROLL)
                        if prev_u is not None:
                            emit_tail_tr(layer, "r", prev_u, prev_u % UNROLL)
                        emit_tail_math(layer, "f", u)
                        emit_mms(layer, "r", is0, u, (u - 1) % UNROLL)
                        emit_xp_inject(layer, i, u + 1)
                        emit_tail_tr(layer, "f", u, u % UNROLL)
                        if layer == 0 and u == 4 and i is not None:
                            flush_half(i * UNROLL - 8, 1)
                        if layer == 0 and u == 12:
                            flush_half(0 if i is None else i * UNROLL, 0)
                        prev_u = u
                    emit_tail_math(layer, "r", prev_u)
                    emit_tail_tr(layer, "r", prev_u, prev_u % UNROLL)

                steps(None, range(PRO))
                with tc.For_i(1, T // UNROLL) as i:
                    steps(i, range(UNROLL))
                if layer == 0:
                    flush_half(T - 8, 1)

            # ================= run =================
            for d in "fr":
                nc.sync.dma_start(out=whh_t[d], in_=whh_d[(0, d)].ap())
            projection(0)
            recurrence(0)
            projection(1)
            for d in "fr":
                nc.sync.dma_start(out=whh_t[d], in_=whh_d[(1, d)].ap())
            recurrence(1)

            # ---------- final linear ----------
            plr = {d: sbuf.tile([128, 4 * B], F32R, name=f"plr_{d}") for d in "fr"}
            for d in "fr":
                nc.vector.tensor_copy(plr[d], pooled[d])
            fin_ps = psum.tile([O, B], F32, tag="gA", name="fin_ps")
            nc.tensor.matmul(fin_ps, lhsT=blin_t, rhs=ones_t[:, 0:B],
                             start=True, stop=False)
            for k in range(8):
                dd = "f" if k < 4 else "r"
                nc.tensor.matmul(
                    fin_ps,
                    lhsT=wlin_t[:, O * k : O * (k + 1)],
                    rhs=plr[dd][:, B * (k % 4) : B * (k % 4 + 1)],
                    start=False, stop=(k == 7))
            fin_sb = sbuf.tile([O, B], F32)
            nc.scalar.copy(fin_sb, fin_ps)
            nc.sync.dma_start(out=out_d.ap(), in_=fin_sb)

    nc.compile()
    return nc


# ======================= host side =======================

def _prep_weights(inputs):
    import ml_dtypes
    f32 = np.float32
    bf16 = ml_dtypes.bfloat16
    perm = np.concatenate([np.arange(512, 1024), np.arange(1536, 2048),
                           np.arange(0, 512), np.arange(1024, 1536)])
    rs = np.ones(G4, f32) * 0.5
    rs[1536:2048] = 1.0

    def whh_dev(W):
        Wp = (W[perm] * rs[:, None] * 0.5).astype(f32)     # [2048, 512]
        return np.ascontiguousarray(
            Wp.T.reshape(4, 128, G4).transpose(1, 0, 2).reshape(128, 4 * G4)).astype(bf16)

    def wih1_dev(W):
        Wp = (W[perm] * rs[:, None] * 0.5).astype(f32)     # [2048, 1024]
        return np.ascontiguousarray(
            Wp.T.reshape(8, 128, G4).transpose(1, 0, 2).reshape(128, 8 * G4)).astype(bf16)

    out = {}
    for d in "fr":
        out[f"wih0{d}"] = np.ascontiguousarray(
            (inputs[f"Wih0{d}"][perm] * rs[:, None]).astype(f32).T).astype(bf16)
        out[f"whh0{d}"] = whh_dev(inputs[f"Whh0{d}"])
        out[f"b0{d}"] = (inputs[f"b0{d}"][perm] * rs).astype(f32)[None, :]
        out[f"wih1{d}"] = wih1_dev(inputs[f"Wih1{d}"])
        out[f"whh1{d}"] = whh_dev(inputs[f"Whh1{d}"])
        out[f"b1{d}"] = (inputs[f"b1{d}"][perm] * rs).astype(f32)[None, :]
    wl = (inputs["Wlin"] * (0.5 / T)).astype(f32)           # [10, 1024]
    out["wlin"] = np.ascontiguousarray(
        wl.T.reshape(8, 128, O).transpose(1, 0, 2).reshape(128, 8 * O))
    out["blin"] = inputs["blin"].astype(f32)[None, :]
    out["ones"] = np.ones((1, 128), f32)
    out["i4"] = np.eye(B, dtype=f32)
    out["i4b"] = np.eye(B, dtype=f32).astype(bf16)
    p836 = np.zeros((8, 36), f32)
    p836[:4, :4] = np.eye(4)
    p836[4:, 32:] = np.eye(4)
    out["i8"] = p836
    return out


def make_in_maps(inputs):
    import ml_dtypes
    shared = _prep_weights(inputs)
    x = np.asarray(inputs["x"], dtype=np.float32)           # [32, 128, 512]
    in_maps = []
    for c in range(NCORES):
        xs = x[B * c : B * (c + 1)]                         # [4, 128, 512]
        m = dict(shared)
        m["xT"] = np.ascontiguousarray(
            xs.transpose(1, 2, 0).reshape(I_IN, T * B)).astype(ml_dtypes.bfloat16)
        in_maps.append(m)
    return in_maps


def kernel(**inputs):
    from concourse.bass_utils import run_bass_kernel_spmd

    if "nc" not in _CACHE:
        _CACHE["nc"] = _build_nc()
    nc = _CACHE["nc"]

    in_maps = make_in_maps(inputs)
    res = run_bass_kernel_spmd(nc, in_maps, core_ids=list(range(NCORES)))
    out = np.zeros((B_FULL, O), np.float32)
    for c in range(NCORES):
        out[B * c : B * (c + 1)] = res.results[c]["out"].T
    return out


# revision 21
# speedup vs baseline: 1.2008x; 1.1925x over previous
"""Trainium2 Bass kernel: 2-layer BiLSTM classifier (B=32, I=128, T=512, H=512, O=10).

Sharding: data-parallel over batch across 8 NeuronCores (b=4 per core); both
directions and both layers run locally per core; host splits/concats.

Per layer per direction:
  xp = Wih' @ input + b'                          (bulk projection -> DRAM)
  per step: g = xp_t + Whh' @ H_{t-1}             (xp DMA'd into PSUM, PE accumulates)
  all-gate tanh trick (host pre-scales i,f,o rows by 0.5):
      t = tanh(g);  for i/f/o blocks t = 2*sigmoid(a)-1; for g block t = tanh(a)
      C_t = 0.5*(t_f+1)*C_{t-1} + (t_i+1)*t_g     [C = 2c]
      H_t = (t_o+1)*tanh(0.5*C_t)                 [H = 2h]
  The H=2h factor is absorbed into Whh/Wih1/Wlin columns (x0.5 host side).
Gate-block order is (f, o, i, g); block = PSUM bank.

PSUM: two parity tiles (gA/gB) of 4 banks each; fwd occupies partitions 0:4,
bwd partitions 4:8 of the same banks.  xp rows for step u+1 are DMA'd into
parity (u+1)%2 while step u computes, and all Whh matmuls run start=False,
accumulating onto the DMA'd xp.  Hidden states stay in SBUF (bf16) and feed
layer-1's projection directly; matmul operands are bf16 (psum accum f32).
"""

import numpy as np

B_FULL, I_IN, T, H, O = 32, 128, 512, 512, 10
NCORES = 8
B = B_FULL // NCORES      # 4
G4 = 4 * H                # 2048
NB = 4                    # gate banks
TBLK = 32                 # projection t-block
NTBLK = T // TBLK         # 16
PRO = 16                  # python-unrolled prologue steps
UNROLL = 16               # steps per For_i iteration
XPAD = UNROLL * B         # xp scratch row padding (prefetch slack both ends)

_CACHE = {}


def _build_nc():
    import concourse.bass as bass
    import concourse.mybir as mybir
    import concourse.tile as tile
    from concourse import bacc
    from concourse.bass import ds

    F32 = mybir.dt.float32
    F32R = mybir.dt.float32r
    BF16 = mybir.dt.bfloat16
    AF = mybir.ActivationFunctionType
    OP = mybir.AluOpType

    nc = bacc.Bacc("TRN2", target_bir_lowering=False, debug=False, num_devices=NCORES)

    # ---------------- I/O ----------------
    xT_d = nc.dram_tensor("xT", [I_IN, T * B], BF16, kind="ExternalInput")
    wih0_d = {d: nc.dram_tensor(f"wih0{d}", [I_IN, G4], BF16, kind="ExternalInput") for d in "fr"}
    wih1_d = {d: nc.dram_tensor(f"wih1{d}", [128, 8 * G4], BF16, kind="ExternalInput") for d in "fr"}
    whh_d = {(l, d): nc.dram_tensor(f"whh{l}{d}", [128, 4 * G4], BF16, kind="ExternalInput")
             for l in range(2) for d in "fr"}
    b_d = {(l, d): nc.dram_tensor(f"b{l}{d}", [1, G4], F32R, kind="ExternalInput")
           for l in range(2) for d in "fr"}
    wlin_d = nc.dram_tensor("wlin", [128, 8 * O], F32R, kind="ExternalInput")
    blin_d = nc.dram_tensor("blin", [1, O], F32R, kind="ExternalInput")
    ones_d = nc.dram_tensor("ones", [1, 128], F32R, kind="ExternalInput")
    i4_d = nc.dram_tensor("i4", [B, B], F32R, kind="ExternalInput")
    i8_d = nc.dram_tensor("i8", [8, 36], F32R, kind="ExternalInput")
    i4b_d = nc.dram_tensor("i4b", [B, B], BF16, kind="ExternalInput")
    out_d = nc.dram_tensor("out", [O, B], F32, kind="ExternalOutput")

    # DRAM scratch (rows padded by XPAD at both ends for prefetch slack)
    xp_dram = {(l, d): nc.dram_tensor(f"xp{l}{d}", [(T + 2 * UNROLL) * B, G4], F32R)
               for l in range(2) for d in "fr"}

    with tile.TileContext(nc) as tc:
        import contextlib

        ctx = contextlib.ExitStack()
        sbuf = ctx.enter_context(tc.tile_pool(name="sbuf", bufs=1))
        psum = ctx.enter_context(tc.tile_pool(name="psum", bufs=1, space="PSUM"))
        xpp = ctx.enter_context(tc.tile_pool(name="xpp", bufs=2))   # brow slots
        tsp = ctx.enter_context(tc.tile_pool(name="tsp", bufs=2))   # tsb slots
        evp = ctx.enter_context(tc.tile_pool(name="evp", bufs=3))   # 2KB slots
        smal = ctx.enter_context(tc.tile_pool(name="smal", bufs=1))
        wpp = ctx.enter_context(tc.tile_pool(name="wpp", bufs=2))   # 8KB slots
        xtp = ctx.enter_context(tc.tile_pool(name="xtp", bufs=2))   # xt slots

        with ctx:
            # ---------- static tiles ----------
            ones_t = sbuf.tile([1, 128], F32R)
            nc.sync.dma_start(out=ones_t, in_=ones_d.ap())
            i4_t = sbuf.tile([B, B], F32R)
            nc.sync.dma_start(out=i4_t, in_=i4_d.ap())
            i8_t = sbuf.tile([8, 36], F32R)
            nc.sync.dma_start(out=i8_t, in_=i8_d.ap())
            i4w_t = sbuf.tile([36, B], BF16, name="i4w")
            nc.sync.dma_start(out=i4w_t[0:B, :], in_=i4b_d.ap())
            nc.sync.dma_start(out=i4w_t[32:36, :], in_=i4b_d.ap())
            blin_t = sbuf.tile([1, O], F32R)
            nc.sync.dma_start(out=blin_t, in_=blin_d.ap())
            wlin_t = sbuf.tile([128, 8 * O], F32R)
            nc.sync.dma_start(out=wlin_t, in_=wlin_d.ap())

            # hring: [128, k(4), slot(8), b(4)] bf16 per dir
            hring = {d: sbuf.tile([128, 4, UNROLL, B], BF16, name=f"hring_{d}") for d in "fr"}
            whh_t = {d: sbuf.tile([128, 4 * G4], BF16, name=f"whh_{d}") for d in "fr"}
            # hidden states of layer 0, SBUF-resident: [128, k(4), T*B] bf16
            hs_sb = {d: sbuf.tile([128, 4, T * B], BF16, name=f"hs_{d}") for d in "fr"}
            c_t = {d: sbuf.tile([36, H], F32, name=f"c_{d}") for d in "fr"}
            pooled = {d: sbuf.tile([128, 4 * B], F32, name=f"pooled_{d}") for d in "fr"}

            ROW = {"f": 0, "r": 32}     # partition offset per direction
            TRC = {"f": 0, "r": 16}    # trv column offset per direction

            # ================= projection =================
            def projection(layer):
                brow = {}
                for d in "fr":
                    brow[d] = xpp.tile([1, G4], F32R, tag=f"xpc{d}", name=f"brow{d}", bufs=1)
                    nc.sync.dma_start(out=brow[d], in_=b_d[(layer, d)].ap())
                if layer == 0:
                    wih0_t = wpp.tile([I_IN, 2 * G4], BF16, tag="wp0", bufs=1)
                    for di, d in enumerate("fr"):
                        nc.sync.dma_start(
                            out=wih0_t[:, G4 * di : G4 * (di + 1)], in_=wih0_d[d].ap())
                for di, d in enumerate("fr"):
                    for blk in range(NB):
                        if layer == 1:
                            wt = wpp.tile([128, 8 * 512], BF16, tag="wp")
                            for k in range(8):
                                nc.sync.dma_start(
                                    out=wt[:, 512 * k : 512 * (k + 1)],
                                    in_=wih1_d[d].ap()[:, G4 * k + 512 * blk : G4 * k + 512 * blk + 512],
                                )
                        for t0 in range(NTBLK):
                            pp = psum.tile([128, 512], F32, tag=("gA" if t0 % 2 == 0 else "gB"), name="pp")
                            nc.tensor.matmul(
                                pp, lhsT=ones_t,
                                rhs=brow[d][:, 512 * blk : 512 * (blk + 1)],
                                start=True, stop=False)
                            if layer == 0:
                                xt = xtp.tile([I_IN, TBLK * B], BF16, tag="xt")
                                nc.sync.dma_start(
                                    out=xt,
                                    in_=xT_d.ap()[:, TBLK * B * t0 : TBLK * B * (t0 + 1)])
                                nc.tensor.matmul(
                                    pp, lhsT=xt,
                                    rhs=wih0_t[:, G4 * di + 512 * blk : G4 * di + 512 * blk + 512],
                                    start=False, stop=True)
                            else:
                                for k in range(8):
                                    dd = "f" if k < 4 else "r"
                                    nc.tensor.matmul(
                                        pp,
                                        lhsT=hs_sb[dd][:, k % 4, TBLK * B * t0 : TBLK * B * t0 + TBLK * B],
                                        rhs=wt[:, 512 * k : 512 * (k + 1)],
                                        start=False, stop=(k == 7))
                            ev = evp.tile([128, 512], F32R, tag="ev")
                            nc.scalar.activation(ev, pp, AF.Identity)
                            nc.sync.dma_start(
                                out=xp_dram[(layer, d)].ap()[
                                    XPAD + TBLK * B * t0 : XPAD + TBLK * B * (t0 + 1),
                                    512 * blk : 512 * (blk + 1)],
                                in_=ev)

            # ================= recurrence =================
            # Two-direction software pipeline: per step the PE FIFO sees
            # [fwd MM-group][bwd tail(t-1) transposes][bwd MM-group][fwd tail]
            # so PE never stalls on a gate chain.
            tsb_cur = {}
            gps = {}

            def toff_of(d, i, u):
                if i is None:
                    t = u
                    tt = t if d == "f" else T - 1 - t
                    return B * tt
                if d == "f":
                    return i * (UNROLL * B) + u * B
                return i * (-UNROLL * B) + (T - 1 - u) * B

            def emit_xp_inject(layer, i, u, is_t0=False):
                # Stage xp rows for step (i, u) into SBUF (both dirs stacked),
                # then one PE inject matmul per bank covers fwd+bwd at once.
                buf = gps[u % 2]
                xpc = xpp.tile([8, G4], F32R, tag="xpc", name="xpc", bufs=2)
                for d in "fr":
                    toff = toff_of(d, i, u)
                    if isinstance(toff, int):
                        src = xp_dram[(layer, d)].ap()[XPAD + toff : XPAD + toff + B, :]
                    else:
                        src = xp_dram[(layer, d)].ap()[ds(XPAD + toff, B), :]
                    r0x = 0 if d == "f" else B
                    nc.sync.dma_start(out=xpc[r0x : r0x + B, :], in_=src)
                for blk in range(NB):
                    nc.tensor.matmul(
                        buf[0:36, blk, :],
                        lhsT=i8_t,
                        rhs=xpc[:, 512 * blk : 512 * (blk + 1)],
                        start=True, stop=is_t0, skip_group_check=True)

            def emit_mms(layer, d, is_t0, u, pslot):
                buf = gps[u % 2]
                r0 = ROW[d]
                tsb = tsp.tile([36, G4], BF16, tag=f"tsb{d}", name=f"tsb{d}")
                tsb_cur[d] = tsb
                if not is_t0:
                    for blk in range(NB):
                        for k in range(4):
                            nc.tensor.matmul(
                                buf[r0 : r0 + B, blk, :],
                                lhsT=hring[d][:, k, pslot, :],
                                rhs=whh_t[d][:, G4 * k + 512 * blk : G4 * k + 512 * blk + 512],
                                start=False, stop=(k == 3), skip_group_check=True)
                nc.scalar.activation(
                    tsb[r0 : r0 + B, 0:1024].rearrange("b (n g) -> b n g", n=2),
                    buf[r0 : r0 + B, 0:2, :], AF.Tanh)
                nc.scalar.activation(
                    tsb[r0 : r0 + B, 1024:2048].rearrange("b (n g) -> b n g", n=2),
                    buf[r0 : r0 + B, 2:4, :], AF.Tanh)

            h_cur = {}

            def emit_tail_math(layer, d, u):
                r0 = ROW[d]
                tsb = tsb_cur[d]
                rr = slice(r0, r0 + B)
                bb_t = smal.tile([36, H], F32, tag="bb", name="bb_t")
                nc.vector.scalar_tensor_tensor(
                    out=bb_t[rr], in0=tsb[rr, 0:512], scalar=1.0,
                    in1=c_t[d][rr], op0=OP.add, op1=OP.mult)
                a_t = smal.tile([36, H], BF16, tag="a", name="a_t")
                nc.vector.scalar_tensor_tensor(
                    out=a_t[rr], in0=tsb[rr, 1024:1536], scalar=1.0,
                    in1=tsb[rr, 1536:2048], op0=OP.add, op1=OP.mult)
                nc.vector.scalar_tensor_tensor(
                    out=c_t[d][rr], in0=bb_t[rr], scalar=0.5,
                    in1=a_t[rr], op0=OP.mult, op1=OP.add)
                tch = smal.tile([36, H], BF16, tag="tc", name="tch")
                nc.scalar.activation(tch[rr], c_t[d][rr], AF.Tanh, scale=0.5)
                h_t = smal.tile([36, H], BF16, tag="h", name="h_t")
                h_cur[d] = h_t
                nc.vector.scalar_tensor_tensor(
                    out=h_t[rr], in0=tsb[rr, 512:1024], scalar=1.0,
                    in1=tch[rr], op0=OP.add, op1=OP.mult)

            def emit_tail_tr(layer, d, u, slot):
                # f-transposes land in the *other* parity tile (pre-inject) so
                # they do not WAR against tanh_r's read of parity u.
                buf = gps[(u + 1) % 2] if d == "f" else gps[u % 2]
                r0 = ROW[d]
                c0 = TRC[d]
                rr = slice(r0, r0 + B)
                h_t = h_cur[d]
                trv = buf.bitcast(BF16)
                for k in range(4):
                    nc.tensor.transpose(
                        trv[:, 0, c0 + B * k : c0 + B * (k + 1)],
                        h_t[rr, 128 * k : 128 * (k + 1)],
                        i4w_t[rr])
                # next-step stationary + layer-0 hs archive ring
                nc.vector.tensor_copy(
                    hring[d][:, :, slot, :],
                    trv[:, 0, c0 : c0 + 16].rearrange("p (k b) -> p k b", k=4))
                if layer == 1:
                    nc.vector.tensor_tensor(
                        out=pooled[d], in0=pooled[d],
                        in1=trv[:, 0, c0 : c0 + 16],
                        op=OP.add)

            def flush_half(t_first, half):
                # archive 8 completed h-slots [half*8 : half*8+8] into hs_sb.
                # t_first = global step index of slot half*8 (int or reg expr).
                HB = 8 * B  # 32 cols per half
                s0 = half * 8
                d = "f"
                base = B * t_first if not isinstance(t_first, int) else B * t_first
                srcap = bass.AP(
                    tensor=hring[d].tensor,
                    offset=hring[d][:, 0, s0, 0:B].offset,
                    ap=[list(hring[d].ap[0]), [UNROLL * B, 4], [1, HB]],
                )
                if isinstance(t_first, int):
                    dst = hs_sb[d][:, :, B * t_first : B * t_first + HB]
                else:
                    dst = hs_sb[d][:, :, ds(B * t_first, HB)]
                nc.sync.dma_start(out=dst, in_=srcap)
                d = "r"
                rbase = B * (T - 8 - t_first)
                for k in range(4):
                    srcap = bass.AP(
                        tensor=hring[d].tensor,
                        offset=hring[d][:, k, s0 + 7, 0:B].offset,
                        ap=[list(hring[d].ap[0]), [-B, 8], [1, B]],
                    )
                    if isinstance(t_first, int):
                        dst = hs_sb[d][:, k, rbase : rbase + HB]
                    else:
                        dst = hs_sb[d][:, k, ds(rbase, HB)]
                    nc.sync.dma_start(out=dst, in_=srcap)

            def recurrence(layer):
                gps[0] = psum.tile([128, NB, 512], F32, tag="gA", name="gA")
                gps[1] = psum.tile([128, NB, 512], F32, tag="gB", name="gB")
                for d in "fr":
                    nc.vector.memset(c_t[d], 0.0)
                    if layer == 1:
                        nc.vector.memset(pooled[d], 0.0)

                def steps(i, urange):
                    prev_u = None
                    for u in urange:
                        is0 = (i is None and u == 0)
                        if is0:
                            emit_xp_inject(layer, i, u, is_t0=True)
                        if prev_u is not None:
                            emit_tail_math(layer, "r", prev_u)
                        emit_mms(layer, "f", is0, u, (u - 1) % UNROLL)
                        if prev_u is not None:
                            emit_tail_tr(layer, "r", prev_u, prev_u % UNROLL)
                        emit_tail_math(layer, "f", u)
                        emit_mms(layer, "r", is0, u, (u - 1) % UNROLL)
                        emit_tail_tr(layer, "f", u, u % UNROLL)
                        emit_xp_inject(layer, i, u + 1)
                        if layer == 0 and u == 4 and i is not None:
                            flush_half(i * UNROLL - 8, 1)
                        if layer == 0 and u == 12:
                            flush_half(0 if i is None else i * UNROLL, 0)
                        prev_u = u
                    emit_tail_math(layer, "r", prev_u)
                    emit_tail_tr(layer, "r", prev_u, prev_u % UNROLL)

                steps(None, range(PRO))
                with tc.For_i(1, T // UNROLL) as i:
                    steps(i, range(UNROLL))
                if layer == 0:
                    flush_half(T - 8, 1)

            # ================= run =================
            for d in "fr":
                nc.sync.dma_start(out=whh_t[d], in_=whh_d[(0, d)].ap())
            projection(0)
            recurrence(0)
            projection(1)
            for d in "fr":
                nc.sync.dma_start(out=whh_t[d], in_=whh_d[(1, d)].ap())
            recurrence(1)

            # ---------- final linear ----------
            plr = {d: sbuf.tile([128, 4 * B], F32R, name=f"plr_{d}") for d in "fr"}
            for d in "fr":
                nc.vector.tensor_copy(plr[d], pooled[d])
            fin_ps = psum.tile([O, B], F32, tag="gA", name="fin_ps")
            nc.tensor.matmul(fin_ps, lhsT=blin_t, rhs=ones_t[:, 0:B],
                             start=True, stop=False)
            for k in range(8):
                dd = "f" if k < 4 else "r"
                nc.tensor.matmul(
                    fin_ps,
                    lhsT=wlin_t[:, O * k : O * (k + 1)],
                    rhs=plr[dd][:, B * (k % 4) : B * (k % 4 + 1)],
                    start=False, stop=(k == 7))
            fin_sb = sbuf.tile([O, B], F32)
            nc.scalar.copy(fin_sb, fin_ps)
            nc.sync.dma_start(out=out_d.ap(), in_=fin_sb)

    nc.compile()
    return nc


# ======================= host side =======================

def _prep_weights(inputs):
    import ml_dtypes
    f32 = np.float32
    bf16 = ml_dtypes.bfloat16
    perm = np.concatenate([np.arange(512, 1024), np.arange(1536, 2048),
                           np.arange(0, 512), np.arange(1024, 1536)])
    rs = np.ones(G4, f32) * 0.5
    rs[1536:2048] = 1.0

    def whh_dev(W):
        Wp = (W[perm] * rs[:, None] * 0.5).astype(f32)     # [2048, 512]
        return np.ascontiguousarray(
            Wp.T.reshape(4, 128, G4).transpose(1, 0, 2).reshape(128, 4 * G4)).astype(bf16)

    def wih1_dev(W):
        Wp = (W[perm] * rs[:, None] * 0.5).astype(f32)     # [2048, 1024]
        return np.ascontiguousarray(
            Wp.T.reshape(8, 128, G4).transpose(1, 0, 2).reshape(128, 8 * G4)).astype(bf16)

    out = {}
    for d in "fr":
        out[f"wih0{d}"] = np.ascontiguousarray(
            (inputs[f"Wih0{d}"][perm] * rs[:, None]).astype(f32).T).astype(bf16)
        out[f"whh0{d}"] = whh_dev(inputs[f"Whh0{d}"])
        out[f"b0{d}"] = (inputs[f"b0{d}"][perm] * rs).astype(f32)[None, :]
        out[f"wih1{d}"] = wih1_dev(inputs[f"Wih1{d}"])
        out[f"whh1{d}"] = whh_dev(inputs[f"Whh1{d}"])
        out[f"b1{d}"] = (inputs[f"b1{d}"][perm] * rs).astype(f32)[None, :]
    wl = (inputs["Wlin"] * (0.5 / T)).astype(f32)           # [10, 1024]
    out["wlin"] = np.ascontiguousarray(
        wl.T.reshape(8, 128, O).transpose(1, 0, 2).reshape(128, 8 * O))
    out["blin"] = inputs["blin"].astype(f32)[None, :]
    out["ones"] = np.ones((1, 128), f32)
    out["i4"] = np.eye(B, dtype=f32)
    out["i4b"] = np.eye(B, dtype=f32).astype(bf16)
    p836 = np.zeros((8, 36), f32)
    p836[:4, :4] = np.eye(4)
    p836[4:, 32:] = np.eye(4)
    out["i8"] = p836
    return out


def make_in_maps(inputs):
    import ml_dtypes
    shared = _prep_weights(inputs)
    x = np.asarray(inputs["x"], dtype=np.float32)           # [32, 128, 512]
    in_maps = []
    for c in range(NCORES):
        xs = x[B * c : B * (c + 1)]                         # [4, 128, 512]
        m = dict(shared)
        m["xT"] = np.ascontiguousarray(
            xs.transpose(1, 2, 0).reshape(I_IN, T * B)).astype(ml_dtypes.bfloat16)
        in_maps.append(m)
    return in_maps


def kernel(**inputs):
    from concourse.bass_utils import run_bass_kernel_spmd

    if "nc" not in _CACHE:
        _CACHE["nc"] = _build_nc()
    nc = _CACHE["nc"]

    in_maps = make_in_maps(inputs)
    res = run_bass_kernel_spmd(nc, in_maps, core_ids=list(range(NCORES)))
    out = np.zeros((B_FULL, O), np.float32)
    for c in range(NCORES):
        out[B * c : B * (c + 1)] = res.results[c]["out"].T
    return out


# revision 26
# speedup vs baseline: 1.3493x; 1.1237x over previous
"""Trainium2 Bass kernel: 2-layer BiLSTM classifier (B=32, I=128, T=512, H=512, O=10).

Sharding: data-parallel over batch across 8 NeuronCores (b=4 per core); both
directions and both layers run locally per core; host splits/concats.

Per layer per direction:
  xp = Wih' @ input + b'                          (bulk projection -> DRAM)
  per step: g = xp_t + Whh' @ H_{t-1}             (xp DMA'd into PSUM, PE accumulates)
  all-gate tanh trick (host pre-scales i,f,o rows by 0.5):
      t = tanh(g);  for i/f/o blocks t = 2*sigmoid(a)-1; for g block t = tanh(a)
      C_t = 0.5*(t_f+1)*C_{t-1} + (t_i+1)*t_g     [C = 2c]
      H_t = (t_o+1)*tanh(0.5*C_t)                 [H = 2h]
  The H=2h factor is absorbed into Whh/Wih1/Wlin columns (x0.5 host side).
Gate-block order is (f, o, i, g); block = PSUM bank.

PSUM: two parity tiles (gA/gB) of 4 banks each; fwd occupies partitions 0:4,
bwd partitions 4:8 of the same banks.  xp rows for step u+1 are DMA'd into
parity (u+1)%2 while step u computes, and all Whh matmuls run start=False,
accumulating onto the DMA'd xp.  Hidden states stay in SBUF (bf16) and feed
layer-1's projection directly; matmul operands are bf16 (psum accum f32).
"""

import numpy as np

B_FULL, I_IN, T, H, O = 32, 128, 512, 512, 10
NCORES = 8
B = B_FULL // NCORES      # 4
G4 = 4 * H                # 2048
NB = 4                    # gate banks
TBLK = 32                 # projection t-block
NTBLK = T // TBLK         # 16
PRO = 16                  # python-unrolled prologue steps
UNROLL = 16               # steps per For_i iteration
XPAD = UNROLL * B         # xp scratch row padding (prefetch slack both ends)

_CACHE = {}


def _build_nc():
    import concourse.bass as bass
    import concourse.mybir as mybir
    import concourse.tile as tile
    from concourse import bacc
    from concourse.bass import ds

    F32 = mybir.dt.float32
    F32R = mybir.dt.float32r
    BF16 = mybir.dt.bfloat16
    AF = mybir.ActivationFunctionType
    OP = mybir.AluOpType

    nc = bacc.Bacc("TRN2", target_bir_lowering=False, debug=False, num_devices=NCORES)

    # ---------------- I/O ----------------
    xT_d = nc.dram_tensor("xT", [I_IN, T * B], BF16, kind="ExternalInput")
    wih0_d = {d: nc.dram_tensor(f"wih0{d}", [I_IN, G4], BF16, kind="ExternalInput") for d in "fr"}
    wih1_d = {d: nc.dram_tensor(f"wih1{d}", [128, 8 * G4], BF16, kind="ExternalInput") for d in "fr"}
    whh_d = {(l, d): nc.dram_tensor(f"whh{l}{d}", [128, 4 * G4], BF16, kind="ExternalInput")
             for l in range(2) for d in "fr"}
    b_d = {(l, d): nc.dram_tensor(f"b{l}{d}", [1, G4], F32R, kind="ExternalInput")
           for l in range(2) for d in "fr"}
    wlin_d = nc.dram_tensor("wlin", [128, 8 * O], F32R, kind="ExternalInput")
    blin_d = nc.dram_tensor("blin", [1, O], F32R, kind="ExternalInput")
    ones_d = nc.dram_tensor("ones", [1, 128], F32R, kind="ExternalInput")
    i4_d = nc.dram_tensor("i4", [B, B], F32R, kind="ExternalInput")
    i4b_d = nc.dram_tensor("i4b", [B, B], BF16, kind="ExternalInput")
    out_d = nc.dram_tensor("out", [O, B], F32, kind="ExternalOutput")

    # DRAM scratch (rows padded by XPAD at both ends for prefetch slack)
    xp_dram = {(l, d): nc.dram_tensor(f"xp{l}{d}", [(T + 2 * UNROLL) * B, G4], F32R)
               for l in range(2) for d in "fr"}

    with tile.TileContext(nc) as tc:
        import contextlib

        ctx = contextlib.ExitStack()
        sbuf = ctx.enter_context(tc.tile_pool(name="sbuf", bufs=1))
        psum = ctx.enter_context(tc.tile_pool(name="psum", bufs=1, space="PSUM"))
        xpp = ctx.enter_context(tc.tile_pool(name="xpp", bufs=2))   # brow slots
        tsp = ctx.enter_context(tc.tile_pool(name="tsp", bufs=2))   # tsb slots
        evp = ctx.enter_context(tc.tile_pool(name="evp", bufs=3))   # 2KB slots
        smal = ctx.enter_context(tc.tile_pool(name="smal", bufs=1))
        wpp = ctx.enter_context(tc.tile_pool(name="wpp", bufs=2))   # 8KB slots
        xtp = ctx.enter_context(tc.tile_pool(name="xtp", bufs=2))   # xt slots

        with ctx:
            # ---------- static tiles ----------
            ones_t = sbuf.tile([1, 128], F32R)
            nc.sync.dma_start(out=ones_t, in_=ones_d.ap())
            i4_t = sbuf.tile([B, B], F32R)
            nc.sync.dma_start(out=i4_t, in_=i4_d.ap())
            i4b_t = sbuf.tile([B, B], BF16, name="i4b")
            nc.sync.dma_start(out=i4b_t, in_=i4b_d.ap())
            blin_t = sbuf.tile([1, O], F32R)
            nc.sync.dma_start(out=blin_t, in_=blin_d.ap())
            wlin_t = sbuf.tile([128, 8 * O], F32R)
            nc.sync.dma_start(out=wlin_t, in_=wlin_d.ap())

            # hring: [128, k(4), slot(8), b(4)] bf16 per dir
            hring = {d: sbuf.tile([128, 4, UNROLL, B], BF16, name=f"hring_{d}") for d in "fr"}
            whh_t = {d: sbuf.tile([128, 4 * G4], BF16, name=f"whh_{d}") for d in "fr"}
            # hidden states of layer 0, SBUF-resident: [128, k(4), T*B] bf16
            hs_sb = {d: sbuf.tile([128, 4, T * B], BF16, name=f"hs_{d}") for d in "fr"}
            c_t = {d: sbuf.tile([B, H], F32, name=f"c_{d}") for d in "fr"}
            xq = {(d, j): sbuf.tile([B, G4], F32R, name=f"xq{d}{j}")
                  for d in "fr" for j in range(2)}
            pooled = {d: sbuf.tile([128, 4 * B], F32, name=f"pooled_{d}") for d in "fr"}



            # ================= projection =================
            def projection(layer):
                brow = {}
                for d in "fr":
                    brow[d] = xpp.tile([1, G4], F32R, tag=f"xpc{d}", name=f"brow{d}", bufs=1)
                    nc.sync.dma_start(out=brow[d], in_=b_d[(layer, d)].ap())
                if layer == 0:
                    wih0_t = wpp.tile([I_IN, 2 * G4], BF16, tag="wp0", bufs=1)
                    for di, d in enumerate("fr"):
                        nc.sync.dma_start(
                            out=wih0_t[:, G4 * di : G4 * (di + 1)], in_=wih0_d[d].ap())
                for di, d in enumerate("fr"):
                    for blk in range(NB):
                        if layer == 1:
                            wt = wpp.tile([128, 8 * 512], BF16, tag="wp")
                            for k in range(8):
                                nc.sync.dma_start(
                                    out=wt[:, 512 * k : 512 * (k + 1)],
                                    in_=wih1_d[d].ap()[:, G4 * k + 512 * blk : G4 * k + 512 * blk + 512],
                                )
                        for t0 in range(NTBLK):
                            pp = psum.tile([128, 512], F32, tag=("gF" if t0 % 2 == 0 else "gR"), name="pp")
                            nc.tensor.matmul(
                                pp, lhsT=ones_t,
                                rhs=brow[d][:, 512 * blk : 512 * (blk + 1)],
                                start=True, stop=False)
                            if layer == 0:
                                xt = xtp.tile([I_IN, TBLK * B], BF16, tag="xt")
                                nc.sync.dma_start(
                                    out=xt,
                                    in_=xT_d.ap()[:, TBLK * B * t0 : TBLK * B * (t0 + 1)])
                                nc.tensor.matmul(
                                    pp, lhsT=xt,
                                    rhs=wih0_t[:, G4 * di + 512 * blk : G4 * di + 512 * blk + 512],
                                    start=False, stop=True)
                            else:
                                for k in range(8):
                                    dd = "f" if k < 4 else "r"
                                    nc.tensor.matmul(
                                        pp,
                                        lhsT=hs_sb[dd][:, k % 4, TBLK * B * t0 : TBLK * B * t0 + TBLK * B],
                                        rhs=wt[:, 512 * k : 512 * (k + 1)],
                                        start=False, stop=(k == 7))
                            ev = evp.tile([128, 512], F32R, tag="ev")
                            nc.scalar.activation(ev, pp, AF.Identity)
                            nc.sync.dma_start(
                                out=xp_dram[(layer, d)].ap()[
                                    XPAD + TBLK * B * t0 : XPAD + TBLK * B * (t0 + 1),
                                    512 * blk : 512 * (blk + 1)],
                                in_=ev)

            # ================= recurrence =================
            # Per-direction PSUM tiles (4 banks each); every gate-tile access
            # is intra-direction, so tanh of one direction never serializes
            # against the other direction's matmuls.  Per step-dir: inject
            # (xp via PE, start=True), 16 Whh matmuls (start=False), 2-part
            # tanh, DVE tail, transposes back into the same tile's bank-0
            # corner (after the tanh read), hring copy, then next inject.
            tsb_cur = {}
            h_cur = {}
            gps = {}

            def toff_of(d, i, u):
                if i is None:
                    t = u
                    tt = t if d == "f" else T - 1 - t
                    return B * tt
                if d == "f":
                    return i * (UNROLL * B) + u * B
                return i * (-UNROLL * B) + (T - 1 - u) * B

            def emit_xp_load(layer, d, i, u):
                xpc = xq[(d, u % 2)]
                toff = toff_of(d, i, u)
                if isinstance(toff, int):
                    srcap = xp_dram[(layer, d)].ap()[XPAD + toff : XPAD + toff + B, :]
                else:
                    srcap = xp_dram[(layer, d)].ap()[ds(XPAD + toff, B), :]
                nc.sync.dma_start(out=xpc, in_=srcap)

            def emit_inject(d, u, is_t0=False):
                xpc = xq[(d, u % 2)]
                for blk in range(NB):
                    nc.tensor.matmul(
                        gps[d][0:B, blk, :],
                        lhsT=i4_t,
                        rhs=xpc[:, 512 * blk : 512 * (blk + 1)],
                        start=True, stop=is_t0, skip_group_check=True)

            def emit_mms(layer, d, is_t0, u, pslot):
                buf = gps[d]
                tsb = tsp.tile([B, G4], BF16, tag=f"tsb{d}", name=f"tsb{d}")
                tsb_cur[d] = tsb
                if not is_t0:
                    for blk in range(NB):
                        for k in range(4):
                            nc.tensor.matmul(
                                buf[0:B, blk, :],
                                lhsT=hring[d][:, k, pslot, :],
                                rhs=whh_t[d][:, G4 * k + 512 * blk : G4 * k + 512 * blk + 512],
                                start=False, stop=(k == 3), skip_group_check=True)
                nc.scalar.activation(
                    tsb[:, 0:1024].rearrange("b (n g) -> b n g", n=2),
                    buf[0:B, 0:2, :], AF.Tanh)
                nc.scalar.activation(
                    tsb[:, 1024:2048].rearrange("b (n g) -> b n g", n=2),
                    buf[0:B, 2:4, :], AF.Tanh)

            def emit_tail_math(layer, d, u):
                tsb = tsb_cur[d]
                bb_t = smal.tile([B, H], F32, tag="bb", name="bb_t")
                nc.vector.scalar_tensor_tensor(
                    out=bb_t, in0=tsb[:, 0:512], scalar=1.0,
                    in1=c_t[d], op0=OP.add, op1=OP.mult)
                a_t = smal.tile([B, H], BF16, tag="a", name="a_t")
                nc.vector.scalar_tensor_tensor(
                    out=a_t, in0=tsb[:, 1024:1536], scalar=1.0,
                    in1=tsb[:, 1536:2048], op0=OP.add, op1=OP.mult)
                nc.vector.scalar_tensor_tensor(
                    out=c_t[d], in0=bb_t, scalar=0.5,
                    in1=a_t, op0=OP.mult, op1=OP.add)
                tch = smal.tile([B, H], BF16, tag="tc", name="tch")
                nc.scalar.activation(tch, c_t[d], AF.Tanh, scale=0.5)
                h_t = smal.tile([B, H], BF16, tag="h", name="h_t")
                h_cur[d] = h_t
                nc.vector.scalar_tensor_tensor(
                    out=h_t, in0=tsb[:, 512:1024], scalar=1.0,
                    in1=tch, op0=OP.add, op1=OP.mult)

            def emit_tail_tr(layer, d, u, slot):
                buf = gps[d]
                h_t = h_cur[d]
                trv = buf.bitcast(BF16)
                for k in range(4):
                    nc.tensor.transpose(
                        trv[:, 0, B * k : B * (k + 1)],
                        h_t[:, 128 * k : 128 * (k + 1)],
                        i4b_t)
                # raw byte move through matching f32 views so the WAR against
                # the next inject (also f32 view) is tracked by Tile
                nc.vector.tensor_copy(
                    hring[d].bitcast(F32)[:, :, slot, :],
                    buf[:, 0, 0:8].rearrange("p (k c) -> p k c", k=4))
                if layer == 1:
                    nc.vector.tensor_tensor(
                        out=pooled[d].rearrange("p (k b) -> p k b", k=4),
                        in0=pooled[d].rearrange("p (k b) -> p k b", k=4),
                        in1=hring[d][:, :, slot, :],
                        op=OP.add)

            def flush_half(t_first, half):
                # archive 8 completed h-slots [half*8 : half*8+8] into hs_sb.
                HB = 8 * B  # 32 cols per half
                s0 = half * 8
                d = "f"
                srcap = bass.AP(
                    tensor=hring[d].tensor,
                    offset=hring[d][:, 0, s0, 0:B].offset,
                    ap=[list(hring[d].ap[0]), [UNROLL * B, 4], [1, HB]],
                )
                if isinstance(t_first, int):
                    dst = hs_sb[d][:, :, B * t_first : B * t_first + HB]
                else:
                    dst = hs_sb[d][:, :, ds(B * t_first, HB)]
                nc.sync.dma_start(out=dst, in_=srcap)
                d = "r"
                rbase = B * (T - 8 - t_first)
                for k in range(4):
                    srcap = bass.AP(
                        tensor=hring[d].tensor,
                        offset=hring[d][:, k, s0 + 7, 0:B].offset,
                        ap=[list(hring[d].ap[0]), [-B, 8], [1, B]],
                    )
                    if isinstance(t_first, int):
                        dst = hs_sb[d][:, k, rbase : rbase + HB]
                    else:
                        dst = hs_sb[d][:, k, ds(rbase, HB)]
                    nc.sync.dma_start(out=dst, in_=srcap)

            def recurrence(layer):
                gps["f"] = psum.tile([128, NB, 512], F32, tag="gF", name="gF")
                gps["r"] = psum.tile([128, NB, 512], F32, tag="gR", name="gR")
                for d in "fr":
                    nc.vector.memset(c_t[d], 0.0)
                    if layer == 1:
                        nc.vector.memset(pooled[d], 0.0)

                def steps(i, urange):
                    prev_u = None
                    for u in urange:
                        is0 = (i is None and u == 0)
                        if is0:
                            emit_xp_load(layer, "f", i, u)
                            emit_inject("f", u, is_t0=True)
                            emit_xp_load(layer, "r", i, u)
                        if prev_u is not None:
                            emit_tail_math(layer, "r", prev_u)
                        emit_mms(layer, "f", is0, u, (u - 1) % UNROLL)
                        if prev_u is not None:
                            emit_tail_tr(layer, "r", prev_u, prev_u % UNROLL)
                        emit_inject("r", u, is_t0=is0)       # xp_r(u) into gps[r]
                        emit_xp_load(layer, "r", i, u + 1)
                        emit_tail_math(layer, "f", u)
                        emit_mms(layer, "r", is0, u, (u - 1) % UNROLL)
                        emit_xp_load(layer, "f", i, u + 1)
                        emit_tail_tr(layer, "f", u, u % UNROLL)
                        emit_inject("f", u + 1)              # xp_f(u+1) into gps[f]
                        if layer == 0 and u == 4 and i is not None:
                            flush_half(i * UNROLL - 8, 1)
                        if layer == 0 and u == 12:
                            flush_half(0 if i is None else i * UNROLL, 0)
                        prev_u = u
                    emit_tail_math(layer, "r", prev_u)
                    emit_tail_tr(layer, "r", prev_u, prev_u % UNROLL)

                steps(None, range(PRO))
                with tc.For_i(1, T // UNROLL) as i:
                    steps(i, range(UNROLL))
                if layer == 0:
                    flush_half(T - 8, 1)

            # ================= run =================
            for d in "fr":
                nc.sync.dma_start(out=whh_t[d], in_=whh_d[(0, d)].ap())
            projection(0)
            recurrence(0)
            projection(1)
            for d in "fr":
                nc.sync.dma_start(out=whh_t[d], in_=whh_d[(1, d)].ap())
            recurrence(1)

            # ---------- final linear ----------
            plr = {d: sbuf.tile([128, 4 * B], F32R, name=f"plr_{d}") for d in "fr"}
            for d in "fr":
                nc.vector.tensor_copy(plr[d], pooled[d])
            fin_ps = psum.tile([O, B], F32, tag="gF", name="fin_ps")
            nc.tensor.matmul(fin_ps, lhsT=blin_t, rhs=ones_t[:, 0:B],
                             start=True, stop=False)
            for k in range(8):
                dd = "f" if k < 4 else "r"
                nc.tensor.matmul(
                    fin_ps,
                    lhsT=wlin_t[:, O * k : O * (k + 1)],
                    rhs=plr[dd][:, B * (k % 4) : B * (k % 4 + 1)],
                    start=False, stop=(k == 7))
            fin_sb = sbuf.tile([O, B], F32)
            nc.scalar.copy(fin_sb, fin_ps)
            nc.sync.dma_start(out=out_d.ap(), in_=fin_sb)

    nc.compile()
    return nc


# ======================= host side =======================

def _prep_weights(inputs):
    import ml_dtypes
    f32 = np.float32
    bf16 = ml_dtypes.bfloat16
    perm = np.concatenate([np.arange(512, 1024), np.arange(1536, 2048),
                           np.arange(0, 512), np.arange(1024, 1536)])
    rs = np.ones(G4, f32) * 0.5
    rs[1536:2048] = 1.0

    def whh_dev(W):
        Wp = (W[perm] * rs[:, None] * 0.5).astype(f32)     # [2048, 512]
        return np.ascontiguousarray(
            Wp.T.reshape(4, 128, G4).transpose(1, 0, 2).reshape(128, 4 * G4)).astype(bf16)

    def wih1_dev(W):
        Wp = (W[perm] * rs[:, None] * 0.5).astype(f32)     # [2048, 1024]
        return np.ascontiguousarray(
            Wp.T.reshape(8, 128, G4).transpose(1, 0, 2).reshape(128, 8 * G4)).astype(bf16)

    out = {}
    for d in "fr":
        out[f"wih0{d}"] = np.ascontiguousarray(
            (inputs[f"Wih0{d}"][perm] * rs[:, None]).astype(f32).T).astype(bf16)
        out[f"whh0{d}"] = whh_dev(inputs[f"Whh0{d}"])
        out[f"b0{d}"] = (inputs[f"b0{d}"][perm] * rs).astype(f32)[None, :]
        out[f"wih1{d}"] = wih1_dev(inputs[f"Wih1{d}"])
        out[f"whh1{d}"] = whh_dev(inputs[f"Whh1{d}"])
        out[f"b1{d}"] = (inputs[f"b1{d}"][perm] * rs).astype(f32)[None, :]
    wl = (inputs["Wlin"] * (0.5 / T)).astype(f32)           # [10, 1024]
    out["wlin"] = np.ascontiguousarray(
        wl.T.reshape(8, 128, O).transpose(1, 0, 2).reshape(128, 8 * O))
    out["blin"] = inputs["blin"].astype(f32)[None, :]
    out["ones"] = np.ones((1, 128), f32)
    out["i4"] = np.eye(B, dtype=f32)
    out["i4b"] = np.eye(B, dtype=f32).astype(bf16)
    return out


def make_in_maps(inputs):
    import ml_dtypes
    shared = _prep_weights(inputs)
    x = np.asarray(inputs["x"], dtype=np.float32)           # [32, 128, 512]
    in_maps = []
    for c in range(NCORES):
        xs = x[B * c : B * (c + 1)]                         # [4, 128, 512]
        m = dict(shared)
        m["xT"] = np.ascontiguousarray(
            xs.transpose(1, 2, 0).reshape(I_IN, T * B)).astype(ml_dtypes.bfloat16)
        in_maps.append(m)
    return in_maps


def kernel(**inputs):
    from concourse.bass_utils import run_bass_kernel_spmd

    if "nc" not in _CACHE:
        _CACHE["nc"] = _build_nc()
    nc = _CACHE["nc"]

    in_maps = make_in_maps(inputs)
    res = run_bass_kernel_spmd(nc, in_maps, core_ids=list(range(NCORES)))
    out = np.zeros((B_FULL, O), np.float32)
    for c in range(NCORES):
        out[B * c : B * (c + 1)] = res.results[c]["out"].T
    return out


# revision 27
# speedup vs baseline: 1.3820x; 1.0243x over previous
"""Trainium2 Bass kernel: 2-layer BiLSTM classifier (B=32, I=128, T=512, H=512, O=10).

Sharding: data-parallel over batch across 8 NeuronCores (b=4 per core); both
directions and both layers run locally per core; host splits/concats.

Per layer per direction:
  xp = Wih' @ input + b'                          (bulk projection -> DRAM)
  per step: g = xp_t + Whh' @ H_{t-1}             (xp DMA'd into PSUM, PE accumulates)
  all-gate tanh trick (host pre-scales i,f,o rows by 0.5):
      t = tanh(g);  for i/f/o blocks t = 2*sigmoid(a)-1; for g block t = tanh(a)
      C_t = 0.5*(t_f+1)*C_{t-1} + (t_i+1)*t_g     [C = 2c]
      H_t = (t_o+1)*tanh(0.5*C_t)                 [H = 2h]
  The H=2h factor is absorbed into Whh/Wih1/Wlin columns (x0.5 host side).
Gate-block order is (f, o, i, g); block = PSUM bank.

PSUM: two parity tiles (gA/gB) of 4 banks each; fwd occupies partitions 0:4,
bwd partitions 4:8 of the same banks.  xp rows for step u+1 are DMA'd into
parity (u+1)%2 while step u computes, and all Whh matmuls run start=False,
accumulating onto the DMA'd xp.  Hidden states stay in SBUF (bf16) and feed
layer-1's projection directly; matmul operands are bf16 (psum accum f32).
"""

import numpy as np

B_FULL, I_IN, T, H, O = 32, 128, 512, 512, 10
NCORES = 8
B = B_FULL // NCORES      # 4
G4 = 4 * H                # 2048
NB = 4                    # gate banks
TBLK = 32                 # projection t-block
NTBLK = T // TBLK         # 16
PRO = 32                  # python-unrolled prologue steps
UNROLL = 32               # steps per For_i iteration
XPAD = UNROLL * B         # xp scratch row padding (prefetch slack both ends)

_CACHE = {}


def _build_nc():
    import concourse.bass as bass
    import concourse.mybir as mybir
    import concourse.tile as tile
    from concourse import bacc
    from concourse.bass import ds

    F32 = mybir.dt.float32
    F32R = mybir.dt.float32r
    BF16 = mybir.dt.bfloat16
    AF = mybir.ActivationFunctionType
    OP = mybir.AluOpType

    nc = bacc.Bacc("TRN2", target_bir_lowering=False, debug=False, num_devices=NCORES)

    # ---------------- I/O ----------------
    xT_d = nc.dram_tensor("xT", [I_IN, T * B], BF16, kind="ExternalInput")
    wih0_d = {d: nc.dram_tensor(f"wih0{d}", [I_IN, G4], BF16, kind="ExternalInput") for d in "fr"}
    wih1_d = {d: nc.dram_tensor(f"wih1{d}", [128, 8 * G4], BF16, kind="ExternalInput") for d in "fr"}
    whh_d = {(l, d): nc.dram_tensor(f"whh{l}{d}", [128, 4 * G4], BF16, kind="ExternalInput")
             for l in range(2) for d in "fr"}
    b_d = {(l, d): nc.dram_tensor(f"b{l}{d}", [1, G4], F32R, kind="ExternalInput")
           for l in range(2) for d in "fr"}
    wlin_d = nc.dram_tensor("wlin", [128, 8 * O], F32R, kind="ExternalInput")
    blin_d = nc.dram_tensor("blin", [1, O], F32R, kind="ExternalInput")
    ones_d = nc.dram_tensor("ones", [1, 128], F32R, kind="ExternalInput")
    i4_d = nc.dram_tensor("i4", [B, B], F32R, kind="ExternalInput")
    i4b_d = nc.dram_tensor("i4b", [B, B], BF16, kind="ExternalInput")
    out_d = nc.dram_tensor("out", [O, B], F32, kind="ExternalOutput")

    # DRAM scratch (rows padded by XPAD at both ends for prefetch slack)
    xp_dram = {(l, d): nc.dram_tensor(f"xp{l}{d}", [(T + 2 * UNROLL) * B, G4], F32R)
               for l in range(2) for d in "fr"}

    with tile.TileContext(nc) as tc:
        import contextlib

        ctx = contextlib.ExitStack()
        sbuf = ctx.enter_context(tc.tile_pool(name="sbuf", bufs=1))
        psum = ctx.enter_context(tc.tile_pool(name="psum", bufs=1, space="PSUM"))
        xpp = ctx.enter_context(tc.tile_pool(name="xpp", bufs=2))   # brow slots
        tsp = ctx.enter_context(tc.tile_pool(name="tsp", bufs=2))   # tsb slots
        evp = ctx.enter_context(tc.tile_pool(name="evp", bufs=4))   # 2KB slots
        smal = ctx.enter_context(tc.tile_pool(name="smal", bufs=1))
        wpp = ctx.enter_context(tc.tile_pool(name="wpp", bufs=2))   # 8KB slots
        xtp = ctx.enter_context(tc.tile_pool(name="xtp", bufs=2))   # xt slots

        with ctx:
            # ---------- static tiles ----------
            ones_t = sbuf.tile([1, 128], F32R)
            nc.sync.dma_start(out=ones_t, in_=ones_d.ap())
            i4_t = sbuf.tile([B, B], F32R)
            nc.sync.dma_start(out=i4_t, in_=i4_d.ap())
            i4b_t = sbuf.tile([B, B], BF16, name="i4b")
            nc.sync.dma_start(out=i4b_t, in_=i4b_d.ap())
            blin_t = sbuf.tile([1, O], F32R)
            nc.sync.dma_start(out=blin_t, in_=blin_d.ap())
            wlin_t = sbuf.tile([128, 8 * O], F32R)
            nc.sync.dma_start(out=wlin_t, in_=wlin_d.ap())

            # hring: [128, k(4), slot(8), b(4)] bf16 per dir
            hring = {d: sbuf.tile([128, 4, UNROLL, B], BF16, name=f"hring_{d}") for d in "fr"}
            whh_t = {d: sbuf.tile([128, 4 * G4], BF16, name=f"whh_{d}") for d in "fr"}
            # hidden states of layer 0, SBUF-resident: [128, k(4), T*B] bf16
            hs_sb = {d: sbuf.tile([128, 4, T * B], BF16, name=f"hs_{d}") for d in "fr"}
            c_t = {d: sbuf.tile([B, H], F32, name=f"c_{d}") for d in "fr"}
            xq = {(d, j): sbuf.tile([B, G4], F32R, name=f"xq{d}{j}")
                  for d in "fr" for j in range(2)}
            pooled = {d: sbuf.tile([128, 4 * B], F32, name=f"pooled_{d}") for d in "fr"}



            # ================= projection =================
            def projection(layer):
                brow = {}
                for d in "fr":
                    brow[d] = xpp.tile([1, G4], F32R, tag=f"xpc{d}", name=f"brow{d}", bufs=1)
                    nc.sync.dma_start(out=brow[d], in_=b_d[(layer, d)].ap())
                if layer == 0:
                    wih0_t = wpp.tile([I_IN, 2 * G4], BF16, tag="wp0", bufs=1)
                    for di, d in enumerate("fr"):
                        nc.sync.dma_start(
                            out=wih0_t[:, G4 * di : G4 * (di + 1)], in_=wih0_d[d].ap())
                ppF = psum.tile([128, NB, 512], F32, tag="gF", name="ppF")
                ppR = psum.tile([128, NB, 512], F32, tag="gR", name="ppR")
                for di, d in enumerate("fr"):
                    for blk in range(NB):
                        if layer == 1:
                            wt = wpp.tile([128, 8 * 512], BF16, tag="wp")
                            for k in range(8):
                                nc.sync.dma_start(
                                    out=wt[:, 512 * k : 512 * (k + 1)],
                                    in_=wih1_d[d].ap()[:, G4 * k + 512 * blk : G4 * k + 512 * blk + 512],
                                )
                        for t0 in range(NTBLK):
                            pp = (ppF if (t0 % 8) < 4 else ppR)[:, t0 % 4, :]
                            nc.tensor.matmul(
                                pp, lhsT=ones_t,
                                rhs=brow[d][:, 512 * blk : 512 * (blk + 1)],
                                start=True, stop=False)
                            if layer == 0:
                                xt = xtp.tile([I_IN, TBLK * B], BF16, tag="xt")
                                nc.sync.dma_start(
                                    out=xt,
                                    in_=xT_d.ap()[:, TBLK * B * t0 : TBLK * B * (t0 + 1)])
                                nc.tensor.matmul(
                                    pp, lhsT=xt,
                                    rhs=wih0_t[:, G4 * di + 512 * blk : G4 * di + 512 * blk + 512],
                                    start=False, stop=True)
                            else:
                                for k in range(8):
                                    dd = "f" if k < 4 else "r"
                                    nc.tensor.matmul(
                                        pp,
                                        lhsT=hs_sb[dd][:, k % 4, TBLK * B * t0 : TBLK * B * t0 + TBLK * B],
                                        rhs=wt[:, 512 * k : 512 * (k + 1)],
                                        start=False, stop=(k == 7))
                            ev = evp.tile([128, 512], F32R, tag="ev")
                            nc.scalar.activation(ev, pp, AF.Identity)
                            nc.sync.dma_start(
                                out=xp_dram[(layer, d)].ap()[
                                    XPAD + TBLK * B * t0 : XPAD + TBLK * B * (t0 + 1),
                                    512 * blk : 512 * (blk + 1)],
                                in_=ev)

            # ================= recurrence =================
            # Per-direction PSUM tiles (4 banks each); every gate-tile access
            # is intra-direction, so tanh of one direction never serializes
            # against the other direction's matmuls.  Per step-dir: inject
            # (xp via PE, start=True), 16 Whh matmuls (start=False), 2-part
            # tanh, DVE tail, transposes back into the same tile's bank-0
            # corner (after the tanh read), hring copy, then next inject.
            tsb_cur = {}
            h_cur = {}
            gps = {}

            def toff_of(d, i, u):
                if i is None:
                    t = u
                    tt = t if d == "f" else T - 1 - t
                    return B * tt
                if d == "f":
                    return i * (UNROLL * B) + u * B
                return i * (-UNROLL * B) + (T - 1 - u) * B

            def emit_xp_load(layer, d, i, u):
                xpc = xq[(d, u % 2)]
                toff = toff_of(d, i, u)
                if isinstance(toff, int):
                    srcap = xp_dram[(layer, d)].ap()[XPAD + toff : XPAD + toff + B, :]
                else:
                    srcap = xp_dram[(layer, d)].ap()[ds(XPAD + toff, B), :]
                nc.sync.dma_start(out=xpc, in_=srcap)

            def emit_inject(d, u, is_t0=False):
                xpc = xq[(d, u % 2)]
                for blk in range(NB):
                    nc.tensor.matmul(
                        gps[d][0:B, blk, :],
                        lhsT=i4_t,
                        rhs=xpc[:, 512 * blk : 512 * (blk + 1)],
                        start=True, stop=is_t0, skip_group_check=True)

            def emit_mms(layer, d, is_t0, u, pslot):
                buf = gps[d]
                tsb = tsp.tile([B, G4], BF16, tag=f"tsb{d}", name=f"tsb{d}")
                tsb_cur[d] = tsb
                if not is_t0:
                    for blk in range(NB):
                        for k in range(4):
                            nc.tensor.matmul(
                                buf[0:B, blk, :],
                                lhsT=hring[d][:, k, pslot, :],
                                rhs=whh_t[d][:, G4 * k + 512 * blk : G4 * k + 512 * blk + 512],
                                start=False, stop=(k == 3), skip_group_check=True)
                nc.scalar.activation(
                    tsb[:, 0:1024].rearrange("b (n g) -> b n g", n=2),
                    buf[0:B, 0:2, :], AF.Tanh)
                nc.scalar.activation(
                    tsb[:, 1024:2048].rearrange("b (n g) -> b n g", n=2),
                    buf[0:B, 2:4, :], AF.Tanh)

            def emit_tail_math(layer, d, u):
                tsb = tsb_cur[d]
                bb_t = smal.tile([B, H], F32, tag="bb", name="bb_t")
                nc.vector.scalar_tensor_tensor(
                    out=bb_t, in0=tsb[:, 0:512], scalar=1.0,
                    in1=c_t[d], op0=OP.add, op1=OP.mult)
                a_t = smal.tile([B, H], BF16, tag="a", name="a_t")
                nc.vector.scalar_tensor_tensor(
                    out=a_t, in0=tsb[:, 1024:1536], scalar=1.0,
                    in1=tsb[:, 1536:2048], op0=OP.add, op1=OP.mult)
                nc.vector.scalar_tensor_tensor(
                    out=c_t[d], in0=bb_t, scalar=0.5,
                    in1=a_t, op0=OP.mult, op1=OP.add)
                tch = smal.tile([B, H], BF16, tag="tc", name="tch")
                nc.scalar.activation(tch, c_t[d], AF.Tanh, scale=0.5)
                h_t = smal.tile([B, H], BF16, tag="h", name="h_t")
                h_cur[d] = h_t
                nc.vector.scalar_tensor_tensor(
                    out=h_t, in0=tsb[:, 512:1024], scalar=1.0,
                    in1=tch, op0=OP.add, op1=OP.mult)

            def emit_tail_tr(layer, d, u, slot):
                buf = gps[d]
                h_t = h_cur[d]
                trv = buf.bitcast(BF16)
                for k in range(4):
                    nc.tensor.transpose(
                        trv[:, 0, B * k : B * (k + 1)],
                        h_t[:, 128 * k : 128 * (k + 1)],
                        i4b_t)
                # raw byte move through matching f32 views so the WAR against
                # the next inject (also f32 view) is tracked by Tile
                nc.vector.tensor_copy(
                    hring[d].bitcast(F32)[:, :, slot, :],
                    buf[:, 0, 0:8].rearrange("p (k c) -> p k c", k=4))
                if layer == 1:
                    nc.vector.tensor_tensor(
                        out=pooled[d].rearrange("p (k b) -> p k b", k=4),
                        in0=pooled[d].rearrange("p (k b) -> p k b", k=4),
                        in1=hring[d][:, :, slot, :],
                        op=OP.add)

            def flush_half(t_first, s0):
                # archive 8 completed h-slots [s0 : s0+8] into hs_sb.
                HB = 8 * B  # 32 cols per group
                d = "f"
                srcap = bass.AP(
                    tensor=hring[d].tensor,
                    offset=hring[d][:, 0, s0, 0:B].offset,
                    ap=[list(hring[d].ap[0]), [UNROLL * B, 4], [1, HB]],
                )
                if isinstance(t_first, int):
                    dst = hs_sb[d][:, :, B * t_first : B * t_first + HB]
                else:
                    dst = hs_sb[d][:, :, ds(B * t_first, HB)]
                nc.sync.dma_start(out=dst, in_=srcap)
                d = "r"
                rbase = B * (T - 8 - t_first)
                for k in range(4):
                    srcap = bass.AP(
                        tensor=hring[d].tensor,
                        offset=hring[d][:, k, s0 + 7, 0:B].offset,
                        ap=[list(hring[d].ap[0]), [-B, 8], [1, B]],
                    )
                    if isinstance(t_first, int):
                        dst = hs_sb[d][:, k, rbase : rbase + HB]
                    else:
                        dst = hs_sb[d][:, k, ds(rbase, HB)]
                    nc.sync.dma_start(out=dst, in_=srcap)

            def recurrence(layer):
                gps["f"] = psum.tile([128, NB, 512], F32, tag="gF", name="gF")
                gps["r"] = psum.tile([128, NB, 512], F32, tag="gR", name="gR")
                for d in "fr":
                    nc.vector.memset(c_t[d], 0.0)
                    if layer == 1:
                        nc.vector.memset(pooled[d], 0.0)

                def steps(i, urange):
                    prev_u = None
                    for u in urange:
                        is0 = (i is None and u == 0)
                        if is0:
                            emit_xp_load(layer, "f", i, u)
                            emit_inject("f", u, is_t0=True)
                            emit_xp_load(layer, "r", i, u)
                        if prev_u is not None:
                            emit_tail_math(layer, "r", prev_u)
                        emit_mms(layer, "f", is0, u, (u - 1) % UNROLL)
                        if prev_u is not None:
                            emit_tail_tr(layer, "r", prev_u, prev_u % UNROLL)
                        emit_inject("r", u, is_t0=is0)       # xp_r(u) into gps[r]
                        emit_xp_load(layer, "r", i, u + 1)
                        emit_tail_math(layer, "f", u)
                        emit_mms(layer, "r", is0, u, (u - 1) % UNROLL)
                        emit_xp_load(layer, "f", i, u + 1)
                        emit_tail_tr(layer, "f", u, u % UNROLL)
                        emit_inject("f", u + 1)              # xp_f(u+1) into gps[f]
                        if layer == 0 and u == 4 and i is not None:
                            flush_half(i * UNROLL - 8, 24)
                        if layer == 0 and u in (12, 20, 28):
                            g8 = u - 12  # 8*(group): u12->grp0, u20->grp1, u28->grp2
                            base = g8 if i is None else i * UNROLL + g8
                            flush_half(base, g8)
                        prev_u = u
                    emit_tail_math(layer, "r", prev_u)
                    emit_tail_tr(layer, "r", prev_u, prev_u % UNROLL)

                steps(None, range(PRO))
                with tc.For_i(1, T // UNROLL) as i:
                    steps(i, range(UNROLL))
                if layer == 0:
                    flush_half(T - 8, 24)

            # ================= run =================
            for d in "fr":
                nc.sync.dma_start(out=whh_t[d], in_=whh_d[(0, d)].ap())
            projection(0)
            recurrence(0)
            projection(1)
            for d in "fr":
                nc.sync.dma_start(out=whh_t[d], in_=whh_d[(1, d)].ap())
            recurrence(1)

            # ---------- final linear ----------
            plr = {d: sbuf.tile([128, 4 * B], F32R, name=f"plr_{d}") for d in "fr"}
            for d in "fr":
                nc.vector.tensor_copy(plr[d], pooled[d])
            fin_ps = psum.tile([O, B], F32, tag="gF", name="fin_ps")
            nc.tensor.matmul(fin_ps, lhsT=blin_t, rhs=ones_t[:, 0:B],
                             start=True, stop=False)
            for k in range(8):
                dd = "f" if k < 4 else "r"
                nc.tensor.matmul(
                    fin_ps,
                    lhsT=wlin_t[:, O * k : O * (k + 1)],
                    rhs=plr[dd][:, B * (k % 4) : B * (k % 4 + 1)],
                    start=False, stop=(k == 7))
            fin_sb = sbuf.tile([O, B], F32)
            nc.scalar.copy(fin_sb, fin_ps)
            nc.sync.dma_start(out=out_d.ap(), in_=fin_sb)

    nc.compile()
    return nc


# ======================= host side =======================

def _prep_weights(inputs):
    import ml_dtypes
    f32 = np.float32
    bf16 = ml_dtypes.bfloat16
    perm = np.concatenate([np.arange(512, 1024), np.arange(1536, 2048),
                           np.arange(0, 512), np.arange(1024, 1536)])
    rs = np.ones(G4, f32) * 0.5
    rs[1536:2048] = 1.0

    def whh_dev(W):
        Wp = (W[perm] * rs[:, None] * 0.5).astype(f32)     # [2048, 512]
        return np.ascontiguousarray(
            Wp.T.reshape(4, 128, G4).transpose(1, 0, 2).reshape(128, 4 * G4)).astype(bf16)

    def wih1_dev(W):
        Wp = (W[perm] * rs[:, None] * 0.5).astype(f32)     # [2048, 1024]
        return np.ascontiguousarray(
            Wp.T.reshape(8, 128, G4).transpose(1, 0, 2).reshape(128, 8 * G4)).astype(bf16)

    out = {}
    for d in "fr":
        out[f"wih0{d}"] = np.ascontiguousarray(
            (inputs[f"Wih0{d}"][perm] * rs[:, None]).astype(f32).T).astype(bf16)
        out[f"whh0{d}"] = whh_dev(inputs[f"Whh0{d}"])
        out[f"b0{d}"] = (inputs[f"b0{d}"][perm] * rs).astype(f32)[None, :]
        out[f"wih1{d}"] = wih1_dev(inputs[f"Wih1{d}"])
        out[f"whh1{d}"] = whh_dev(inputs[f"Whh1{d}"])
        out[f"b1{d}"] = (inputs[f"b1{d}"][perm] * rs).astype(f32)[None, :]
    wl = (inputs["Wlin"] * (0.5 / T)).astype(f32)           # [10, 1024]
    out["wlin"] = np.ascontiguousarray(
        wl.T.reshape(8, 128, O).transpose(1, 0, 2).reshape(128, 8 * O))
    out["blin"] = inputs["blin"].astype(f32)[None, :]
    out["ones"] = np.ones((1, 128), f32)
    out["i4"] = np.eye(B, dtype=f32)
    out["i4b"] = np.eye(B, dtype=f32).astype(bf16)
    return out


def make_in_maps(inputs):
    import ml_dtypes
    shared = _prep_weights(inputs)
    x = np.asarray(inputs["x"], dtype=np.float32)           # [32, 128, 512]
    in_maps = []
    for c in range(NCORES):
        xs = x[B * c : B * (c + 1)]                         # [4, 128, 512]
        m = dict(shared)
        m["xT"] = np.ascontiguousarray(
            xs.transpose(1, 2, 0).reshape(I_IN, T * B)).astype(ml_dtypes.bfloat16)
        in_maps.append(m)
    return in_maps


def kernel(**inputs):
    from concourse.bass_utils import run_bass_kernel_spmd

    if "nc" not in _CACHE:
        _CACHE["nc"] = _build_nc()
    nc = _CACHE["nc"]

    in_maps = make_in_maps(inputs)
    res = run_bass_kernel_spmd(nc, in_maps, core_ids=list(range(NCORES)))
    out = np.zeros((B_FULL, O), np.float32)
    for c in range(NCORES):
        out[B * c : B * (c + 1)] = res.results[c]["out"].T
    return out


# revision 29
# speedup vs baseline: 1.3838x; 1.0013x over previous
"""Trainium2 Bass kernel: 2-layer BiLSTM classifier (B=32, I=128, T=512, H=512, O=10).

Sharding: data-parallel over batch across 8 NeuronCores (b=4 per core); both
directions and both layers run locally per core; host splits/concats.

Per layer per direction:
  xp = Wih' @ input + b'                          (bulk projection -> DRAM)
  per step: g = xp_t + Whh' @ H_{t-1}             (xp DMA'd into PSUM, PE accumulates)
  all-gate tanh trick (host pre-scales i,f,o rows by 0.5):
      t = tanh(g);  for i/f/o blocks t = 2*sigmoid(a)-1; for g block t = tanh(a)
      C_t = 0.5*(t_f+1)*C_{t-1} + (t_i+1)*t_g     [C = 2c]
      H_t = (t_o+1)*tanh(0.5*C_t)                 [H = 2h]
  The H=2h factor is absorbed into Whh/Wih1/Wlin columns (x0.5 host side).
Gate-block order is (f, o, i, g); block = PSUM bank.

PSUM: two parity tiles (gA/gB) of 4 banks each; fwd occupies partitions 0:4,
bwd partitions 4:8 of the same banks.  xp rows for step u+1 are DMA'd into
parity (u+1)%2 while step u computes, and all Whh matmuls run start=False,
accumulating onto the DMA'd xp.  Hidden states stay in SBUF (bf16) and feed
layer-1's projection directly; matmul operands are bf16 (psum accum f32).
"""

import numpy as np

B_FULL, I_IN, T, H, O = 32, 128, 512, 512, 10
NCORES = 8
B = B_FULL // NCORES      # 4
G4 = 4 * H                # 2048
NB = 4                    # gate banks
TBLK = 32                 # projection t-block
NTBLK = T // TBLK         # 16
PRO = 32                  # python-unrolled prologue steps
UNROLL = 32               # steps per For_i iteration
XPAD = UNROLL * B         # xp scratch row padding (prefetch slack both ends)

_CACHE = {}


def _build_nc():
    import concourse.bass as bass
    import concourse.mybir as mybir
    import concourse.tile as tile
    from concourse import bacc
    from concourse.bass import ds

    F32 = mybir.dt.float32
    F32R = mybir.dt.float32r
    BF16 = mybir.dt.bfloat16
    AF = mybir.ActivationFunctionType
    OP = mybir.AluOpType

    nc = bacc.Bacc("TRN2", target_bir_lowering=False, debug=False, num_devices=NCORES)

    # ---------------- I/O ----------------
    xT_d = nc.dram_tensor("xT", [I_IN, T * B], BF16, kind="ExternalInput")
    wih0_d = {d: nc.dram_tensor(f"wih0{d}", [I_IN, G4], BF16, kind="ExternalInput") for d in "fr"}
    wih1_d = {d: nc.dram_tensor(f"wih1{d}", [128, 8 * G4], BF16, kind="ExternalInput") for d in "fr"}
    whh_d = {(l, d): nc.dram_tensor(f"whh{l}{d}", [128, 4 * G4], BF16, kind="ExternalInput")
             for l in range(2) for d in "fr"}
    b_d = {(l, d): nc.dram_tensor(f"b{l}{d}", [1, G4], F32R, kind="ExternalInput")
           for l in range(2) for d in "fr"}
    wlin_d = nc.dram_tensor("wlin", [128, 8 * O], F32R, kind="ExternalInput")
    blin_d = nc.dram_tensor("blin", [1, O], F32R, kind="ExternalInput")
    ones_d = nc.dram_tensor("ones", [1, 128], F32R, kind="ExternalInput")
    i4_d = nc.dram_tensor("i4", [B, B], F32R, kind="ExternalInput")
    i4b_d = nc.dram_tensor("i4b", [B, B], BF16, kind="ExternalInput")
    out_d = nc.dram_tensor("out", [O, B], F32, kind="ExternalOutput")

    # DRAM scratch (rows padded by XPAD at both ends for prefetch slack)
    xp_dram = {(l, d): nc.dram_tensor(f"xp{l}{d}", [(T + 2 * UNROLL) * B, G4], F32R)
               for l in range(2) for d in "fr"}

    with tile.TileContext(nc) as tc:
        import contextlib

        ctx = contextlib.ExitStack()
        sbuf = ctx.enter_context(tc.tile_pool(name="sbuf", bufs=1))
        psum = ctx.enter_context(tc.tile_pool(name="psum", bufs=1, space="PSUM"))
        xpp = ctx.enter_context(tc.tile_pool(name="xpp", bufs=2))   # brow slots
        tsp = ctx.enter_context(tc.tile_pool(name="tsp", bufs=2))   # tsb slots
        evp = ctx.enter_context(tc.tile_pool(name="evp", bufs=4))   # 2KB slots
        smal = ctx.enter_context(tc.tile_pool(name="smal", bufs=1))
        wpp = ctx.enter_context(tc.tile_pool(name="wpp", bufs=2))   # 8KB slots
        xtp = ctx.enter_context(tc.tile_pool(name="xtp", bufs=2))   # xt slots

        with ctx:
            # ---------- static tiles ----------
            ones_t = sbuf.tile([1, 128], F32R)
            nc.sync.dma_start(out=ones_t, in_=ones_d.ap())
            i4_t = sbuf.tile([B, B], F32R)
            nc.sync.dma_start(out=i4_t, in_=i4_d.ap())
            i4b_t = sbuf.tile([B, B], BF16, name="i4b")
            nc.sync.dma_start(out=i4b_t, in_=i4b_d.ap())
            blin_t = sbuf.tile([1, O], F32R)
            nc.sync.dma_start(out=blin_t, in_=blin_d.ap())
            wlin_t = sbuf.tile([128, 8 * O], F32R)
            nc.sync.dma_start(out=wlin_t, in_=wlin_d.ap())

            # hring: [128, k(4), slot(8), b(4)] bf16 per dir
            hring = {d: sbuf.tile([128, 4, UNROLL, B], BF16, name=f"hring_{d}") for d in "fr"}
            whh_t = {d: sbuf.tile([128, 4 * G4], BF16, name=f"whh_{d}") for d in "fr"}
            # hidden states of layer 0, SBUF-resident: [128, k(4), T*B] bf16
            hs_sb = {d: sbuf.tile([128, 4, T * B], BF16, name=f"hs_{d}") for d in "fr"}
            c_t = {d: sbuf.tile([B, H], F32, name=f"c_{d}") for d in "fr"}
            xq = {(d, j): sbuf.tile([B, G4], F32R, name=f"xq{d}{j}")
                  for d in "fr" for j in range(2)}
            pooled = {d: sbuf.tile([128, 4 * B], F32, name=f"pooled_{d}") for d in "fr"}



            # ================= projection =================
            def projection(layer):
                brow = {}
                for d in "fr":
                    brow[d] = xpp.tile([1, G4], F32R, tag=f"xpc{d}", name=f"brow{d}", bufs=1)
                    nc.sync.dma_start(out=brow[d], in_=b_d[(layer, d)].ap())
                if layer == 0:
                    wih0_t = wpp.tile([I_IN, 2 * G4], BF16, tag="wp0", bufs=1)
                    for di, d in enumerate("fr"):
                        nc.sync.dma_start(
                            out=wih0_t[:, G4 * di : G4 * (di + 1)], in_=wih0_d[d].ap())
                ppF = psum.tile([128, NB, 512], F32, tag="gF", name="ppF")
                ppR = psum.tile([128, NB, 512], F32, tag="gR", name="ppR")
                for di, d in enumerate("fr"):
                    for blk in range(NB):
                        if layer == 1:
                            wt = wpp.tile([128, 8 * 512], BF16, tag="wp")
                            for k in range(8):
                                nc.sync.dma_start(
                                    out=wt[:, 512 * k : 512 * (k + 1)],
                                    in_=wih1_d[d].ap()[:, G4 * k + 512 * blk : G4 * k + 512 * blk + 512],
                                )
                        for t0 in range(NTBLK):
                            pp = (ppF if (t0 % 8) < 4 else ppR)[:, t0 % 4, :]
                            nc.tensor.matmul(
                                pp, lhsT=ones_t,
                                rhs=brow[d][:, 512 * blk : 512 * (blk + 1)],
                                start=True, stop=False)
                            if layer == 0:
                                xt = xtp.tile([I_IN, TBLK * B], BF16, tag="xt")
                                nc.sync.dma_start(
                                    out=xt,
                                    in_=xT_d.ap()[:, TBLK * B * t0 : TBLK * B * (t0 + 1)])
                                nc.tensor.matmul(
                                    pp, lhsT=xt,
                                    rhs=wih0_t[:, G4 * di + 512 * blk : G4 * di + 512 * blk + 512],
                                    start=False, stop=True)
                            else:
                                for k in range(8):
                                    dd = "f" if k < 4 else "r"
                                    nc.tensor.matmul(
                                        pp,
                                        lhsT=hs_sb[dd][:, k % 4, TBLK * B * t0 : TBLK * B * t0 + TBLK * B],
                                        rhs=wt[:, 512 * k : 512 * (k + 1)],
                                        start=False, stop=(k == 7))
                            ev = evp.tile([128, 512], F32R, tag="ev")
                            nc.scalar.activation(ev, pp, AF.Identity)
                            nc.sync.dma_start(
                                out=xp_dram[(layer, d)].ap()[
                                    XPAD + TBLK * B * t0 : XPAD + TBLK * B * (t0 + 1),
                                    512 * blk : 512 * (blk + 1)],
                                in_=ev)

            # ================= recurrence =================
            # Per-direction PSUM tiles (4 banks each); every gate-tile access
            # is intra-direction, so tanh of one direction never serializes
            # against the other direction's matmuls.  Per step-dir: inject
            # (xp via PE, start=True), 16 Whh matmuls (start=False), 2-part
            # tanh, DVE tail, transposes back into the same tile's bank-0
            # corner (after the tanh read), hring copy, then next inject.
            tsb_cur = {}
            h_cur = {}
            gps = {}

            def toff_of(d, i, u):
                if i is None:
                    t = u
                    tt = t if d == "f" else T - 1 - t
                    return B * tt
                if d == "f":
                    return i * (UNROLL * B) + u * B
                return i * (-UNROLL * B) + (T - 1 - u) * B

            def emit_xp_load(layer, d, i, u):
                xpc = xq[(d, u % 2)]
                toff = toff_of(d, i, u)
                if isinstance(toff, int):
                    srcap = xp_dram[(layer, d)].ap()[XPAD + toff : XPAD + toff + B, :]
                else:
                    srcap = xp_dram[(layer, d)].ap()[ds(XPAD + toff, B), :]
                nc.sync.dma_start(out=xpc, in_=srcap)

            def emit_inject(d, u, is_t0=False):
                xpc = xq[(d, u % 2)]
                for blk in range(NB):
                    nc.tensor.matmul(
                        gps[d][0:B, blk, :],
                        lhsT=i4_t,
                        rhs=xpc[:, 512 * blk : 512 * (blk + 1)],
                        start=True, stop=is_t0, skip_group_check=True)

            def emit_mms(layer, d, is_t0, u, pslot):
                buf = gps[d]
                tsb = tsp.tile([B, G4], BF16, tag=f"tsb{d}", name=f"tsb{d}")
                tsb_cur[d] = tsb
                if not is_t0:
                    for blk in range(NB):
                        for k in range(4):
                            nc.tensor.matmul(
                                buf[0:B, blk, :],
                                lhsT=hring[d][:, k, pslot, :],
                                rhs=whh_t[d][:, G4 * k + 512 * blk : G4 * k + 512 * blk + 512],
                                start=False, stop=(k == 3), skip_group_check=True)
                nc.scalar.activation(
                    tsb[:, 0:1024].rearrange("b (n g) -> b n g", n=2),
                    buf[0:B, 0:2, :], AF.Tanh)
                nc.scalar.activation(
                    tsb[:, 1024:2048].rearrange("b (n g) -> b n g", n=2),
                    buf[0:B, 2:4, :], AF.Tanh)

            def emit_tail_math(layer, d, u):
                tsb = tsb_cur[d]
                bb_t = smal.tile([B, H], F32, tag="bb", name="bb_t")
                nc.vector.scalar_tensor_tensor(
                    out=bb_t, in0=tsb[:, 0:512], scalar=1.0,
                    in1=c_t[d], op0=OP.add, op1=OP.mult)
                a_t = smal.tile([B, H], BF16, tag="a", name="a_t")
                nc.vector.scalar_tensor_tensor(
                    out=a_t, in0=tsb[:, 1024:1536], scalar=1.0,
                    in1=tsb[:, 1536:2048], op0=OP.add, op1=OP.mult)
                nc.vector.scalar_tensor_tensor(
                    out=c_t[d], in0=bb_t, scalar=0.5,
                    in1=a_t, op0=OP.mult, op1=OP.add)
                tch = smal.tile([B, H], BF16, tag="tc", name="tch")
                nc.scalar.activation(tch, c_t[d], AF.Tanh, scale=0.5)
                h_t = smal.tile([B, H], BF16, tag="h", name="h_t")
                h_cur[d] = h_t
                nc.vector.scalar_tensor_tensor(
                    out=h_t, in0=tsb[:, 512:1024], scalar=1.0,
                    in1=tch, op0=OP.add, op1=OP.mult)

            def emit_tail_tr(layer, d, u, slot):
                buf = gps[d]
                h_t = h_cur[d]
                trv = buf.bitcast(BF16)
                for k in range(4):
                    nc.tensor.transpose(
                        trv[:, 0, B * k : B * (k + 1)],
                        h_t[:, 128 * k : 128 * (k + 1)],
                        i4b_t)
                # raw byte move through matching f32 views so the WAR against
                # the next inject (also f32 view) is tracked by Tile
                nc.vector.tensor_copy(
                    hring[d].bitcast(F32)[:, :, slot, :],
                    buf[:, 0, 0:8].rearrange("p (k c) -> p k c", k=4))
                if layer == 1:
                    nc.vector.tensor_tensor(
                        out=pooled[d].rearrange("p (k b) -> p k b", k=4),
                        in0=pooled[d].rearrange("p (k b) -> p k b", k=4),
                        in1=hring[d][:, :, slot, :],
                        op=OP.add)

            def flush_half(t_first, s0):
                # archive 8 completed h-slots [s0 : s0+8] into hs_sb.
                HB = 8 * B  # 32 cols per group
                d = "f"
                srcap = bass.AP(
                    tensor=hring[d].tensor,
                    offset=hring[d][:, 0, s0, 0:B].offset,
                    ap=[list(hring[d].ap[0]), [UNROLL * B, 4], [1, HB]],
                )
                if isinstance(t_first, int):
                    dst = hs_sb[d][:, :, B * t_first : B * t_first + HB]
                else:
                    dst = hs_sb[d][:, :, ds(B * t_first, HB)]
                nc.sync.dma_start(out=dst, in_=srcap)
                d = "r"
                rbase = B * (T - 8 - t_first)
                for k in range(4):
                    srcap = bass.AP(
                        tensor=hring[d].tensor,
                        offset=hring[d][:, k, s0 + 7, 0:B].offset,
                        ap=[list(hring[d].ap[0]), [-B, 8], [1, B]],
                    )
                    if isinstance(t_first, int):
                        dst = hs_sb[d][:, k, rbase : rbase + HB]
                    else:
                        dst = hs_sb[d][:, k, ds(rbase, HB)]
                    nc.sync.dma_start(out=dst, in_=srcap)

            def recurrence(layer):
                gps["f"] = psum.tile([128, NB, 512], F32, tag="gF", name="gF")
                gps["r"] = psum.tile([128, NB, 512], F32, tag="gR", name="gR")
                for d in "fr":
                    nc.vector.memset(c_t[d], 0.0)
                    if layer == 1:
                        nc.vector.memset(pooled[d], 0.0)

                def steps(i, urange):
                    prev_u = None
                    for u in urange:
                        is0 = (i is None and u == 0)
                        if is0:
                            emit_xp_load(layer, "f", i, u)
                            emit_inject("f", u, is_t0=True)
                            emit_xp_load(layer, "r", i, u)
                        if prev_u is not None:
                            emit_tail_math(layer, "r", prev_u)
                        emit_mms(layer, "f", is0, u, (u - 1) % UNROLL)
                        if prev_u is not None:
                            emit_tail_tr(layer, "r", prev_u, prev_u % UNROLL)
                        emit_inject("r", u, is_t0=is0)       # xp_r(u) into gps[r]
                        emit_xp_load(layer, "r", i, u + 1)
                        emit_tail_math(layer, "f", u)
                        emit_mms(layer, "r", is0, u, (u - 1) % UNROLL)
                        emit_xp_load(layer, "f", i, u + 1)
                        emit_tail_tr(layer, "f", u, u % UNROLL)
                        emit_inject("f", u + 1)              # xp_f(u+1) into gps[f]
                        if layer == 0 and u == 4 and i is not None:
                            flush_half(i * UNROLL - 8, 24)
                        if layer == 0 and u in (12, 20, 28):
                            g8 = u - 12  # 8*(group): u12->grp0, u20->grp1, u28->grp2
                            base = g8 if i is None else i * UNROLL + g8
                            flush_half(base, g8)
                        prev_u = u
                    emit_tail_math(layer, "r", prev_u)
                    emit_tail_tr(layer, "r", prev_u, prev_u % UNROLL)

                steps(None, range(PRO))
                with tc.For_i(1, T // UNROLL) as i:
                    steps(i, range(UNROLL))
                if layer == 0:
                    flush_half(T - 8, 24)

            # ================= run =================
            for d in "fr":
                nc.sync.dma_start(out=whh_t[d], in_=whh_d[(0, d)].ap())
            projection(0)
            recurrence(0)
            projection(1)
            for d in "fr":
                nc.sync.dma_start(out=whh_t[d], in_=whh_d[(1, d)].ap())
            recurrence(1)

            # ---------- final linear ----------
            plr = {d: sbuf.tile([128, 4 * B], F32R, name=f"plr_{d}") for d in "fr"}
            for d in "fr":
                nc.vector.tensor_copy(plr[d], pooled[d])
            fin_ps = psum.tile([O, B], F32, tag="gF", name="fin_ps")
            nc.tensor.matmul(fin_ps, lhsT=blin_t, rhs=ones_t[:, 0:B],
                             start=True, stop=False)
            for k in range(8):
                dd = "f" if k < 4 else "r"
                nc.tensor.matmul(
                    fin_ps,
                    lhsT=wlin_t[:, O * k : O * (k + 1)],
                    rhs=plr[dd][:, B * (k % 4) : B * (k % 4 + 1)],
                    start=False, stop=(k == 7))
            fin_sb = sbuf.tile([O, B], F32)
            nc.scalar.copy(fin_sb, fin_ps)
            nc.sync.dma_start(out=out_d.ap(), in_=fin_sb)

    nc.compile()
    return nc


# ======================= host side =======================

def _prep_weights(inputs):
    import ml_dtypes
    f32 = np.float32
    bf16 = ml_dtypes.bfloat16
    perm = np.concatenate([np.arange(512, 1024), np.arange(1536, 2048),
                           np.arange(0, 512), np.arange(1024, 1536)])
    rs = np.ones(G4, f32) * 0.5
    rs[1536:2048] = 1.0

    def whh_dev(W):
        Wp = (W[perm] * rs[:, None] * 0.5).astype(f32)     # [2048, 512]
        return np.ascontiguousarray(
            Wp.T.reshape(4, 128, G4).transpose(1, 0, 2).reshape(128, 4 * G4)).astype(bf16)

    def wih1_dev(W):
        Wp = (W[perm] * rs[:, None] * 0.5).astype(f32)     # [2048, 1024]
        return np.ascontiguousarray(
            Wp.T.reshape(8, 128, G4).transpose(1, 0, 2).reshape(128, 8 * G4)).astype(bf16)

    out = {}
    for d in "fr":
        out[f"wih0{d}"] = np.ascontiguousarray(
            (inputs[f"Wih0{d}"][perm] * rs[:, None]).astype(f32).T).astype(bf16)
        out[f"whh0{d}"] = whh_dev(inputs[f"Whh0{d}"])
        out[f"b0{d}"] = (inputs[f"b0{d}"][perm] * rs).astype(f32)[None, :]
        out[f"wih1{d}"] = wih1_dev(inputs[f"Wih1{d}"])
        out[f"whh1{d}"] = whh_dev(inputs[f"Whh1{d}"])
        out[f"b1{d}"] = (inputs[f"b1{d}"][perm] * rs).astype(f32)[None, :]
    wl = (inputs["Wlin"] * (0.5 / T)).astype(f32)           # [10, 1024]
    out["wlin"] = np.ascontiguousarray(
        wl.T.reshape(8, 128, O).transpose(1, 0, 2).reshape(128, 8 * O))
    out["blin"] = inputs["blin"].astype(f32)[None, :]
    out["ones"] = np.ones((1, 128), f32)
    out["i4"] = np.eye(B, dtype=f32)
    out["i4b"] = np.eye(B, dtype=f32).astype(bf16)
    return out


def make_in_maps(inputs):
    import ml_dtypes
    shared = _prep_weights(inputs)
    x = np.asarray(inputs["x"], dtype=np.float32)           # [32, 128, 512]
    in_maps = []
    for c in range(NCORES):
        xs = x[B * c : B * (c + 1)]                         # [4, 128, 512]
        m = dict(shared)
        m["xT"] = np.ascontiguousarray(
            xs.transpose(1, 2, 0).reshape(I_IN, T * B)).astype(ml_dtypes.bfloat16)
        in_maps.append(m)
    return in_maps


def kernel(**inputs):
    from concourse.bass_utils import run_bass_kernel_spmd

    if "nc" not in _CACHE:
        _CACHE["nc"] = _build_nc()
    nc = _CACHE["nc"]

    in_maps = make_in_maps(inputs)
    res = run_bass_kernel_spmd(nc, in_maps, core_ids=list(range(NCORES)))
    out = np.zeros((B_FULL, O), np.float32)
    for c in range(NCORES):
        out[B * c : B * (c + 1)] = res.results[c]["out"].T
    return out
